# revision 15
# baseline (speedup 1.0000x reference)
"""Trainium2 Bass kernel for nn_Cross_Attention_Global (sparse_attention).

Key algebraic identity: the [512, 4096] score block per (query sample b,
kv sample s) is a product of rank-64 factors, so contract over the embed
dim c=64 FIRST:

  scores_{b,s} = q_b^T (emb_s Wk) = (q_b^T emb_s) Wk = A_s Wk
  ctx_b = sum_s emb_s (Wv attn_{b,s}^T)^T = sum_s emb_s G_s^T

This cuts the big matmuls ~6x vs the direct form (1.34 GFLOP instead of
8.6 GFLOP per plane and per path).

Math per direction d (query sample b owned by this core, kv half KV):
  q_b = Q[b] @ Wq                                   [2048, 512]
  A^T[s*64+c, i] = sum_n KV[s,n,c] q_b[n,i]         [512, 512]
  scores[i, s*512+i'] = sum_c A^T[s*64+c, i] Wk[c,i']
  InstanceNorm over plane + softmax over j: the per-plane mean cancels in
  softmax, so only inv_sigma = rsqrt(var+eps) matters; scores*inv_sigma is
  bounded (|z| < ~7) so exp needs no row-max subtraction either.
  attn = exp(scores * inv_sigma); denom = rowsum (via exp accum_out)
  G^T[s*64+c, i] = sum_i' Wv[c,i'] attnT[(s,i'), i]  [512, 512]
  ctxT[i, n] = sum_{s,c} G^T[s*64+c, i] KV[s,n,c]    (/denom at eviction)
  out_b = ctxT^T @ Wo                                [2048, 64]

Sharding: core c handles query sample c of direction 0 (q=emb_l, kv=emb_u)
and query sample c of direction 1 (q=emb_u, kv=emb_l).  No collectives.

All matmul operands bf16 (fp32 PSUM accumulation); emb/weights converted
to bf16 on the host.  End-to-end rel-err ~6e-3 vs the 2e-2 gate.

Per-core structure, per direction:
  ph1:  q projection (row-tiled pairs of K=64 matmuls)
  ph2a: A^T, 4 PSUM banks (g = s-pairs) accumulating over 16 n-chunks
  ph2b: scores: single-shot K=64 row-tiled pair matmuls (s even/odd in
        AT partition halves); evictions split ACT/DVE; bn_stats on the
        bf16 scores for the plane variance
  ph3:  bn_aggr -> inv_sigma (cross-partition totals via PE ones matmul);
        ACT table pre-loads hide the Sqrt/Exp table switches
  ph4+5a (per s-pair): exp chunks [128,1024] (+denoms via accum_out),
        per-it transpose batches -> attnT, then G^T matmuls per s
  ph5b: ctxT = sum_g GT_g^T @ embSC_g per (i-chunk, n-block); 1/denom
        folded into the eviction
  ph6:  Wo projection, chunked output DMA
Direction 1's ph1+ph2a fill direction 0's scores-eviction/stats window;
direction 1's ph2b fills the gap before ph5b(d0).
"""

import sys

sys.path.insert(0, "/opt/trn_rl_repo")

import numpy as np
import ml_dtypes

_CACHE = {}
REPEAT = 1   # timing knob: execute the whole computation REPEAT times

B = 8          # samples per half-batch
N = 2048       # sequence length
C = 64         # embed dim
CH = 512       # head dim total (q/k/v channels)
J = B * CH     # 4096, kv concat width
NT = N // 128  # 16 n-tiles
IT = CH // 128  # 4 i-tiles
EPS = 1e-5


def _emit(nc, tc, bass, mybir, t):
    f32 = mybir.dt.float32
    bf16 = mybir.dt.bfloat16
    X = mybir.AxisListType
    AF = mybir.ActivationFunctionType
    ALU = mybir.AluOpType

    embT = t["embT"]    # [64, 16*2048] bf16, col = sglobal*2048+n
    embTq = t["embTq"]  # [64, 2*2048] bf16, per-core q samples (dir0, dir1)
    embN = t["embN"]    # [2048, 2*512] bf16, col = d*512 + s*64+c
    out_d = t["out"]    # [2, 2048, 64] f32

    embN_r = embN.rearrange("(k p) s -> p k s", p=128)  # [128, 16, 1024]

    with (
        tc.tile_pool(name="sb", bufs=1) as sb,
        tc.tile_pool(name="ps", bufs=8, space=bass.MemorySpace.PSUM) as ps,
    ):
        def acc():
            return ps.tile([128, 512], f32, tag="acc", bufs=4, name="acc")

        def bank():
            return ps.tile([128, 512], f32, tag="bank", bufs=4, name="bank")

        # --- persistent weights (Wq/Wk duplicated into both partition
        # halves so row-tiled pair matmuls can source at base_partition 64)
        Wq_s = sb.tile([128, 512], bf16)
        nc.sync.dma_start(Wq_s[0:64, :], t["Wq"])
        nc.sync.dma_start(Wq_s[64:128, :], t["Wq"])

        eps_t = sb.tile([1, 1], f32, tag="eps_t")
        nc.gpsimd.memset(eps_t[:], float(EPS))
        ones_t = sb.tile([128, 1], f32, tag="ones_t")
        nc.gpsimd.memset(ones_t[:], 1.0)
        dmy = sb.tile([1, 2], f32, tag="dmy")
        nc.gpsimd.memset(dmy[:], 1.0)

        st = {}  # per-dir tile state

        def ph1(d):
            """q projection for dir d + embN prefetch for dir d."""
            qoff = d * N
            q_s = sb.tile([128, NT * 512], bf16, tag="q_s", name="q_s")
            eqs = sb.tile([128, N], bf16, tag="eqs", name="eqs")
            nc.sync.dma_start(eqs[0:64, :], embTq[:, qoff:qoff + N])
            nc.sync.dma_start(eqs[64:128, :], embTq[:, qoff:qoff + N])
            for nk in range(0, NT, 2):
                qp1, qp2 = bank(), bank()
                nc.tensor.matmul(
                    qp1[:, :], eqs[0:64, nk * 128:(nk + 1) * 128],
                    Wq_s[0:64, :], start=True, stop=True,
                    tile_position=(0, 0),
                )
                nc.tensor.matmul(
                    qp2[:, :], eqs[64:128, (nk + 1) * 128:(nk + 2) * 128],
                    Wq_s[64:128, :], start=True, stop=True,
                    tile_position=(64, 0),
                )
                nc.vector.tensor_copy(q_s[:, nk * 512:(nk + 1) * 512],
                                      qp1[:, :])
                nc.scalar.copy(q_s[:, (nk + 1) * 512:(nk + 2) * 512],
                               qp2[:, :])
            st.setdefault(d, {})["q_s"] = q_s

        def prefetch_eN(d):
            """n-major kv embedding for ph2a(d): 2 MB DMA, issued early."""
            eN = sb.tile([128, NT * 512], bf16, tag="eN", bufs=2, name="eN")
            eN3 = eN[:].rearrange("p (k s) -> p k s", s=512)
            for k0 in range(0, NT, 4):   # chunked: ph2a starts on chunk 0
                nc.sync.dma_start(
                    eN3[:, k0:k0 + 4, :],
                    embN_r[:, k0:k0 + 4, d * 512:(d + 1) * 512],
                )
            st.setdefault(d, {})["eN"] = eN

        def ph2a(d):
            """A^T[g] = sum_n embN_chunk^T @ q: 4 banks over 16 n-chunks."""
            q_s, eN = st[d]["q_s"], st[d]["eN"]
            ab = [acc() for _ in range(4)]
            for nk in range(NT):
                for g in range(4):
                    nc.tensor.matmul(
                        ab[g][:, :],
                        eN[:, nk * 512 + g * 128:nk * 512 + (g + 1) * 128],
                        q_s[:, nk * 512:(nk + 1) * 512],
                        start=(nk == 0), stop=(nk == NT - 1),
                    )
            AT = sb.tile([128, 4 * 512], bf16, tag="AT", bufs=2, name="AT")
            u = sb.tile([128, 4], f32, tag="u", name="u")
            for g in range(4):
                # accum_out gives u[sc] = sum_i A^T[sc, i] for the mean path
                nc.scalar.activation(AT[:, g * 512:(g + 1) * 512],
                                     ab[g][:, :], AF.Copy,
                                     accum_out=u[:, g:g + 1])
            # B^T[(s,c), i] = sum_c' L[c',c] A^T[(s,c'), i]; sum B^2 = sum S^2
            # (K_hat = Wk Wk^T = L L^T, L from the host)
            bst = sb.tile([128, 4 * 6], f32, tag="bst", name="bst")
            for g in range(4):
                bb = bank()
                for z in range(2):
                    nc.tensor.matmul(
                        bb[z * 64:(z + 1) * 64, :],
                        L_s[z * 64:(z + 1) * 64, :],
                        AT[z * 64:(z + 1) * 64, g * 512:(g + 1) * 512],
                        start=True, stop=True,
                        tile_position=(z * 64, z * 64),
                    )
                nc.vector.bn_stats(bst[:, g * 6:(g + 1) * 6], bb[:, :])
            st[d]["AT"] = AT
            st[d]["u"] = u
            st[d]["bst"] = bst

        def ph2b(d):
            """scores block (ic, s) = AT_slice^T @ Wk as row-tiled pairs
            (s even at partitions 0-64, s odd at 64-128)."""
            AT = st[d]["AT"]
            scores_s = sb.tile([128, IT * J], bf16, tag="S", name="scores_s")
            stats = sb.tile([128, IT * B * 6], f32, tag="st", name="stats")
            # pre-load the Sqrt ACT table off the critical path
            nc.scalar.activation(dmy[:, 1:2], dmy[:, 0:1], AF.Sqrt)
            for g in range(4):
                for ic in range(IT):
                    p1, p2 = bank(), bank()
                    nc.tensor.matmul(
                        p1[:, :], AT[0:64, g * 512 + ic * 128:
                                     g * 512 + (ic + 1) * 128],
                        Wk_s[0:64, :], start=True, stop=True,
                        tile_position=(0, 0),
                    )
                    nc.tensor.matmul(
                        p2[:, :], AT[64:128, g * 512 + ic * 128:
                                     g * 512 + (ic + 1) * 128],
                        Wk_s[64:128, :], start=True, stop=True,
                        tile_position=(64, 0),
                    )
                    for z, pz in ((0, p1), (1, p2)):
                        s = 2 * g + z
                        dst = scores_s[:, ic * J + s * 512:
                                       ic * J + (s + 1) * 512]
                        if (ic * 2 + z) % 4 == 3:
                            nc.vector.tensor_copy(dst, pz[:, :])
                        else:
                            nc.scalar.copy(dst, pz[:, :])
                        # plane stats from the bf16 scores (2x DVE mode)
                        col = ic * B + s
                        nc.vector.bn_stats(stats[:, col * 6:(col + 1) * 6],
                                           dst)
            st[d]["scores_s"] = scores_s
            st[d]["stats"] = stats

        def ph3(d):
            """plane variance -> inv_sigma from the B-statistics (sum S^2 =
            sum B^2) and the u/wbar dot (sum S).  Exp bias is not needed
            (softmax shift-invariance) and |s*inv| < ~7 so no row-max."""
            u, bst = st[d]["u"], st[d]["bst"]
            kvs = (1 - d) * B
            esc = st[d].setdefault("esc", [])
            while len(esc) < 2:  # prefetch first embSC tiles for ph5b
                esc.append(_load_esc(kvs, len(esc)))

            bsa = sb.tile([128, 2], f32, tag="cs", name="bsa")
            nc.vector.bn_aggr(bsa[:], bst[:])
            cstat = sb.tile([128, 2], f32, tag="cstat")
            # sumsqB_row = (varB + meanB^2) * 2048
            nc.vector.scalar_tensor_tensor(
                cstat[:, 1:2], bsa[:, 0:1], bsa[:, 0:1], bsa[:, 1:2],
                op0=ALU.mult, op1=ALU.add,
            )
            nc.vector.tensor_scalar_mul(cstat[:, 1:2], cstat[:, 1:2],
                                        float(4 * 512))
            # msum_row = sum_g u[r,g] * wbar[r]
            mp = sb.tile([128, 4], f32, tag="mp", name="mp")
            nc.vector.tensor_scalar(mp[:], u[:], wb_s[:], 1.0,
                                    op0=ALU.mult, op1=ALU.mult)
            nc.vector.reduce_sum(cstat[:, 0:1], mp[:], axis=X.X)
            # cross-partition totals via PE ones-vector matmul
            pst = bank()
            nc.tensor.matmul(pst[:1, :2], ones_t[:], cstat[:],
                             start=True, stop=True)
            tstat = sb.tile([1, 2], f32, tag="tstat")
            nc.vector.tensor_copy(tstat[:], pst[:1, :2])
            mean = sb.tile([1, 1], f32, tag="mean")
            ex2 = sb.tile([1, 1], f32, tag="ex2")
            INVM = 1.0 / float(CH * J)
            nc.vector.tensor_scalar_mul(mean[:], tstat[:, 0:1], INVM)
            nc.vector.tensor_scalar_mul(ex2[:], tstat[:, 1:2], INVM)
            negvar = sb.tile([1, 1], f32, tag="negvar")
            nc.vector.scalar_tensor_tensor(
                negvar[:], mean[:], mean[:], ex2[:],
                op0=ALU.mult, op1=ALU.subtract,
            )
            sig = sb.tile([1, 1], f32, tag="sig")
            # sqrt(-negvar + eps) = sqrt(var + eps); table pre-loaded
            nc.scalar.activation(sig[:], negvar[:], AF.Sqrt,
                                 bias=eps_t[:], scale=-1.0)
            # pre-load the Exp table while DVE finishes the chain
            nc.scalar.activation(dmy[:, 1:2], dmy[:, 0:1], AF.Exp)
            inv = sb.tile([1, 1], f32, tag="inv")
            nc.vector.reciprocal(inv[:], sig[:])
            invb = sb.tile([128, 1], f32, tag="invb")
            nc.gpsimd.partition_broadcast(invb[:], inv[:])
            st[d]["invb"] = invb

        def _load_esc(kvs, g):
            """embSC tile g: samples 2g (rows 0-64) and 2g+1 (64-128)."""
            esc = sb.tile([128, N], bf16, tag="eS", bufs=4, name="esc")
            nc.sync.dma_start(
                esc[0:64, :],
                embT[:, (kvs + 2 * g) * N:(kvs + 2 * g + 1) * N])
            nc.sync.dma_start(
                esc[64:128, :],
                embT[:, (kvs + 2 * g + 1) * N:(kvs + 2 * g + 2) * N])
            return esc

        def ph45a(d):
            """exp + transpose per s-pair, G^T matmuls per s."""
            kvs = (1 - d) * B
            scores_s = st[d]["scores_s"]
            invb = st[d]["invb"]
            attn_s = sb.tile([128, IT * J], bf16, tag="A", name="attn_s")
            # attnT layout: col = jk*512 + it*128 + ii (jk-major)
            attnT_s = sb.tile([128, IT * J], bf16, tag="T", name="attnT_s")
            aT3 = attnT_s.rearrange("p (j c) -> p j c", c=512)
            dparts = sb.tile([128, IT * 4], f32, tag="dp", name="dparts")
            GT = sb.tile([128, 4 * 512], bf16, tag="GT", name="GT")

            esc = st[d]["esc"]
            gb = {}
            for sp in range(4):  # s-pairs (2sp, 2sp+1)
                s0 = 2 * sp
                if sp < 2:  # finish the embSC prefetches for ph5b
                    esc.append(_load_esc(kvs, sp + 2))
                for it in range(IT):
                    nc.scalar.activation(
                        attn_s[:, it * J + s0 * 512:it * J + (s0 + 2) * 512],
                        scores_s[:, it * J + s0 * 512:
                                 it * J + (s0 + 2) * 512],
                        AF.Exp,
                        scale=invb[:],
                        accum_out=dparts[:, it * 4 + sp:it * 4 + sp + 1],
                    )
                for it in range(IT):
                    # 8 transposes (jk = 4*s0 .. 4*s0+7) batched per it
                    tp = ps.tile([128, 1024], bf16, tag="bank", bufs=4,
                                 name="tp")
                    for z in range(8):
                        jk = s0 * 4 + z
                        nc.tensor.transpose(
                            tp[:, z * 128:(z + 1) * 128],
                            attn_s[:, it * J + jk * 128:
                                   it * J + (jk + 1) * 128],
                            eye_s[:],
                        )
                    nc.vector.tensor_copy(
                        aT3[:, s0 * 4:s0 * 4 + 8, it * 128:(it + 1) * 128],
                        tp[:].rearrange("p (j c) -> p j c", c=128),
                    )
                # G^T for s0, s0+1: accumulate into bank sp's halves
                gb[sp] = acc()
                for z in range(2):
                    s = s0 + z
                    for ip in range(4):
                        jk = s * 4 + ip
                        nc.tensor.matmul(
                            gb[sp][z * 64:(z + 1) * 64, :],
                            WvT_s[:, ip * 64:(ip + 1) * 64],
                            attnT_s[:, jk * 512:(jk + 1) * 512],
                            start=(ip == 0), stop=(ip == 3),
                            tile_position=(0, z * 64),
                        )
                # DVE only: keeps the ACT queue a pure exp stream here
                nc.vector.tensor_copy(GT[:, sp * 512:(sp + 1) * 512],
                                      gb[sp][:, :])

            denom = sb.tile([128, IT], f32, tag="denom")
            rden = sb.tile([128, IT], f32, tag="rden")
            for it in range(IT):
                nc.vector.reduce_sum(denom[:, it:it + 1],
                                     dparts[:, it * 4:(it + 1) * 4], axis=X.X)
            nc.vector.reciprocal(rden[:], denom[:])
            st[d]["GT"] = GT
            st[d]["rden"] = rden

        def ph5b(d):
            """ctxT[ic, nt] = sum_g GT_g(ic)^T @ embSC_g; /denom folded into
            the eviction."""
            GT, rden, esc = st[d]["GT"], st[d]["rden"], st[d]["esc"]
            ctxT_s = sb.tile([128, IT * 4 * 512], bf16, tag="A",
                             name="ctxT_s")
            for ic in range(IT):
                for nt in range(4):
                    cp = acc()
                    for g in range(4):
                        nc.tensor.matmul(
                            cp[:, :],
                            GT[:, g * 512 + ic * 128:g * 512 + (ic + 1) * 128],
                            esc[g][:, nt * 512:(nt + 1) * 512],
                            start=(g == 0), stop=(g == 3),
                        )
                    dst = ctxT_s[:, (ic * 4 + nt) * 512:
                                 (ic * 4 + nt + 1) * 512]
                    if nt & 1:
                        nc.vector.tensor_scalar_mul(dst, cp[:, :],
                                                    rden[:, ic:ic + 1])
                    else:
                        nc.scalar.activation(dst, cp[:, :], AF.Copy,
                                             scale=rden[:, ic:ic + 1])
            st[d]["ctxT_s"] = ctxT_s

        def ph6(d):
            """output projection + chunked DMA."""
            ctxT_s = st[d]["ctxT_s"]
            out_s = sb.tile([128, 16 * 64], f32, tag="out_s", name="out_s")
            odst = out_d[d].rearrange("(g p) c -> p g c", p=128)
            osrc = out_s[:].rearrange("p (g c) -> p g c", g=NT)
            for g in range(NT):
                nt, sub = g >> 2, g & 3
                op = acc()
                for it in range(IT):
                    nc.tensor.matmul(
                        op[:, :64],
                        ctxT_s[:, (it * 4 + nt) * 512 + sub * 128:
                               (it * 4 + nt) * 512 + (sub + 1) * 128],
                        Wo_s[:, it * 64:(it + 1) * 64],
                        start=(it == 0), stop=(it == IT - 1),
                    )
                if g & 1:
                    nc.scalar.copy(out_s[:, g * 64:(g + 1) * 64], op[:, :64])
                else:
                    nc.vector.tensor_copy(out_s[:, g * 64:(g + 1) * 64],
                                          op[:, :64])
                if g & 3 == 3:
                    nc.sync.dma_start(odst[:, g - 3:g + 1, :],
                                      osrc[:, g - 3:g + 1, :])

        import contextlib
        loop_cm = (tc.For_i(0, REPEAT, 1) if REPEAT > 1
                   else contextlib.nullcontext())
        with loop_cm:
            prefetch_eN(0)
            ph1(0)
            # remaining persistent weights, behind ph1's inputs on the queue
            Wk_s = sb.tile([128, 512], bf16)
            nc.sync.dma_start(Wk_s[0:64, :], t["Wk"])
            nc.sync.dma_start(Wk_s[64:128, :], t["Wk"])
            WvT_s = sb.tile([128, IT * 64], bf16)
            wvt_r = t["WvT"].rearrange("(k p) c -> k p c", p=128)
            for i in range(IT):
                nc.sync.dma_start(WvT_s[:, i * 64:(i + 1) * 64], wvt_r[i])
            Wo_s = sb.tile([128, IT * 64], bf16)
            wo_r = t["Wo"].rearrange("(t p) c -> t p c", p=128)
            for i in range(IT):
                nc.sync.dma_start(Wo_s[:, i * 64:(i + 1) * 64], wo_r[i])
            eye_s = sb.tile([128, 128], bf16)
            nc.sync.dma_start(eye_s[:], t["eye"])

            ph2a(0)
            prefetch_eN(1)   # 2 MB DMA runs under ph2b(0)+ph1(1)
            ph2b(0)
            ph1(1)    # fills d0's scores-eviction/stats window
            ph2a(1)
            ph3(0)
            ph45a(0)
            ph2b(1)   # fills the gap before ph5b(d0)
            ph5b(0)
            ph6(0)
            ph3(1)
            ph45a(1)
            ph5b(1)
            ph6(1)


def _build():
    key = ("nc", REPEAT)
    if key in _CACHE:
        return _CACHE[key]
    from concourse import bass, bacc, tile, mybir

    f32 = mybir.dt.float32
    bf16 = mybir.dt.bfloat16

    nc = bacc.Bacc("TRN2", target_bir_lowering=False, debug=False)
    t = {
        "embT": nc.dram_tensor("embT", [C, 2 * B * N], bf16,
                               kind="ExternalInput").ap(),
        "embTq": nc.dram_tensor("embTq", [C, 2 * N], bf16,
                                kind="ExternalInput").ap(),
        "embN": nc.dram_tensor("embN", [N, 2 * B * C], bf16,
                               kind="ExternalInput").ap(),
        "Wq": nc.dram_tensor("Wq", [C, CH], bf16, kind="ExternalInput").ap(),
        "Wk": nc.dram_tensor("Wk", [C, CH], bf16, kind="ExternalInput").ap(),
        "WvT": nc.dram_tensor("WvT", [CH, C], bf16,
                              kind="ExternalInput").ap(),
        "Wo": nc.dram_tensor("Wo", [CH, C], bf16, kind="ExternalInput").ap(),
        "eye": nc.dram_tensor("eye", [128, 128], bf16,
                              kind="ExternalInput").ap(),
        "out": nc.dram_tensor("out", [2, N, C], f32,
                              kind="ExternalOutput").ap(),
    }
    with tile.TileContext(nc) as tc:
        _emit(nc, tc, bass, mybir, t)
    nc.compile()
    _CACHE[key] = nc
    return nc


def kernel(emb, Wq, Wk, Wv, Wo):
    from concourse.bass_utils import run_bass_kernel_spmd

    bf = ml_dtypes.bfloat16
    emb = np.asarray(emb, dtype=np.float32)
    Wq = np.ascontiguousarray(np.asarray(Wq, dtype=np.float32)).astype(bf)
    Wk = np.ascontiguousarray(np.asarray(Wk, dtype=np.float32)).astype(bf)
    WvT = np.ascontiguousarray(
        np.asarray(Wv, dtype=np.float32).T).astype(bf)
    Wo = np.ascontiguousarray(np.asarray(Wo, dtype=np.float32)).astype(bf)

    nc = _build()

    embT = np.ascontiguousarray(
        emb.transpose(2, 0, 1).reshape(C, 2 * B * N)).astype(bf)
    # embN[n, d*512 + s*64+c] = emb[kv_half(d) sample s, n, c]
    embN = np.empty((N, 2 * B * C), dtype=np.float32)
    embN[:, 0:B * C] = emb[B:].transpose(1, 0, 2).reshape(N, B * C)
    embN[:, B * C:] = emb[:B].transpose(1, 0, 2).reshape(N, B * C)
    embN = np.ascontiguousarray(embN).astype(bf)
    eye = np.eye(128, dtype=bf)

    in_maps = []
    for c in range(B):
        embTq = np.ascontiguousarray(
            np.concatenate([embT[:, c * N:(c + 1) * N],
                            embT[:, (B + c) * N:(B + c + 1) * N]], axis=1))
        in_maps.append({
            "embT": embT, "embTq": embTq, "embN": embN,
            "Wq": Wq, "Wk": Wk, "WvT": WvT, "Wo": Wo, "eye": eye,
        })

    r = run_bass_kernel_spmd(nc, in_maps, list(range(B)))
    res = r.results

    out = np.empty((2 * B, N, C), dtype=np.float32)
    for c in range(B):
        out[c] = res[c]["out"][0]
        out[B + c] = res[c]["out"][1]
    return out


# revision 20
# speedup vs baseline: 1.2118x; 1.2118x over previous
"""Trainium2 Bass kernel for nn_Cross_Attention_Global (sparse_attention).

Key algebraic identity: the [512, 4096] score block per (query sample b,
kv sample s) is a product of rank-64 factors, so contract over the embed
dim c=64 FIRST:

  scores_{b,s} = q_b^T (emb_s Wk) = (q_b^T emb_s) Wk = A_s Wk
  ctx_b = sum_s emb_s (Wv attn_{b,s}^T)^T = sum_s emb_s G_s^T

This cuts the big matmuls ~6x vs the direct form (1.34 GFLOP instead of
8.6 GFLOP per plane and per path).

Math per direction d (query sample b owned by this core, kv half KV):
  q_b = Q[b] @ Wq                                   [2048, 512]
  A^T[s*64+c, i] = sum_n KV[s,n,c] q_b[n,i]         [512, 512]
  scores[i, s*512+i'] = sum_c A^T[s*64+c, i] Wk[c,i']
  InstanceNorm over plane + softmax over j: the per-plane mean cancels in
  softmax, so only inv_sigma = rsqrt(var+eps) matters; scores*inv_sigma is
  bounded (|z| < ~7) so exp needs no row-max subtraction either.
  attn = exp(scores * inv_sigma); denom = rowsum (via exp accum_out)
  G^T[s*64+c, i] = sum_i' Wv[c,i'] attnT[(s,i'), i]  [512, 512]
  ctxT[i, n] = sum_{s,c} G^T[s*64+c, i] KV[s,n,c]    (/denom at eviction)
  out_b = ctxT^T @ Wo                                [2048, 64]

Sharding: core c handles query sample c of direction 0 (q=emb_l, kv=emb_u)
and query sample c of direction 1 (q=emb_u, kv=emb_l).  No collectives.

All matmul operands bf16 (fp32 PSUM accumulation); emb/weights converted
to bf16 on the host.  End-to-end rel-err ~6e-3 vs the 2e-2 gate.

Per-core structure, per direction:
  ph1:  q projection (row-tiled pairs of K=64 matmuls)
  ph2a: A^T, 4 PSUM banks (g = s-pairs) accumulating over 16 n-chunks
  ph2b: scores: single-shot K=64 row-tiled pair matmuls (s even/odd in
        AT partition halves); evictions split ACT/DVE; bn_stats on the
        bf16 scores for the plane variance
  ph3:  bn_aggr -> inv_sigma (cross-partition totals via PE ones matmul);
        ACT table pre-loads hide the Sqrt/Exp table switches
  ph4+5a (per s-pair): exp chunks [128,1024] (+denoms via accum_out),
        per-it transpose batches -> attnT, then G^T matmuls per s
  ph5b: ctxT = sum_g GT_g^T @ embSC_g per (i-chunk, n-block); 1/denom
        folded into the eviction
  ph6:  Wo projection, chunked output DMA
Direction 1's ph1+ph2a fill direction 0's scores-eviction/stats window;
direction 1's ph2b fills the gap before ph5b(d0).
"""

import sys

sys.path.insert(0, "/opt/trn_rl_repo")

import numpy as np
import ml_dtypes

_CACHE = {}
REPEAT = 1   # timing knob: execute the whole computation REPEAT times

B = 8          # samples per half-batch
N = 2048       # sequence length
C = 64         # embed dim
CH = 512       # head dim total (q/k/v channels)
J = B * CH     # 4096, kv concat width
NT = N // 128  # 16 n-tiles
IT = CH // 128  # 4 i-tiles
EPS = 1e-5


def _emit(nc, tc, bass, mybir, t):
    f32 = mybir.dt.float32
    bf16 = mybir.dt.bfloat16
    X = mybir.AxisListType
    AF = mybir.ActivationFunctionType
    ALU = mybir.AluOpType

    embT = t["embT"]    # [64, 16*2048] bf16, col = sglobal*2048+n
    embTq = t["embTq"]  # [64, 2*2048] bf16, per-core q samples (dir0, dir1)
    embN = t["embN"]    # [2048, 2*512] bf16, col = d*512 + s*64+c
    out_d = t["out"]    # [2, 2048, 64] f32

    embN_r = embN.rearrange("(k p) s -> p k s", p=128)  # [128, 16, 1024]

    with (
        tc.tile_pool(name="sb", bufs=1) as sb,
        tc.tile_pool(name="ps", bufs=8, space=bass.MemorySpace.PSUM) as ps,
    ):
        def acc():
            return ps.tile([128, 512], f32, tag="acc", bufs=4, name="acc")

        def bank():
            return ps.tile([128, 512], f32, tag="bank", bufs=4, name="bank")

        # --- persistent weights (Wq/Wk duplicated into both partition
        # halves so row-tiled pair matmuls can source at base_partition 64)
        Wq_s = sb.tile([128, 512], bf16)
        nc.sync.dma_start(Wq_s[0:64, :], t["Wq"])
        nc.sync.dma_start(Wq_s[64:128, :], t["Wq"])

        eps_t = sb.tile([1, 1], f32, tag="eps_t")
        nc.gpsimd.memset(eps_t[:], float(EPS))
        ones_t = sb.tile([128, 1], f32, tag="ones_t")
        nc.gpsimd.memset(ones_t[:], 1.0)
        dmy = sb.tile([1, 2], f32, tag="dmy")
        nc.gpsimd.memset(dmy[:], 1.0)

        st = {}  # per-dir tile state

        def ph1(d):
            """q projection for dir d + embN prefetch for dir d."""
            qoff = d * N
            q_s = sb.tile([128, NT * 512], bf16, tag="q_s", name="q_s")
            eqs = sb.tile([128, N], bf16, tag="eqs", name="eqs")
            nc.sync.dma_start(eqs[0:64, :], embTq[:, qoff:qoff + N])
            nc.sync.dma_start(eqs[64:128, :], embTq[:, qoff:qoff + N])
            for nk in range(0, NT, 2):
                qp1, qp2 = bank(), bank()
                nc.tensor.matmul(
                    qp1[:, :], eqs[0:64, nk * 128:(nk + 1) * 128],
                    Wq_s[0:64, :], start=True, stop=True,
                    tile_position=(0, 0),
                )
                nc.tensor.matmul(
                    qp2[:, :], eqs[64:128, (nk + 1) * 128:(nk + 2) * 128],
                    Wq_s[64:128, :], start=True, stop=True,
                    tile_position=(64, 0),
                )
                nc.vector.tensor_copy(q_s[:, nk * 512:(nk + 1) * 512],
                                      qp1[:, :])
                nc.scalar.copy(q_s[:, (nk + 1) * 512:(nk + 2) * 512],
                               qp2[:, :])
            st.setdefault(d, {})["q_s"] = q_s

        def prefetch_eN(d):
            """n-major kv embedding for ph2a(d): 2 MB DMA, issued early."""
            eN = sb.tile([128, NT * 512], bf16, tag="eN", bufs=2, name="eN")
            eN3 = eN[:].rearrange("p (k s) -> p k s", s=512)
            for k0 in range(0, NT, 4):   # chunked: ph2a starts on chunk 0
                nc.sync.dma_start(
                    eN3[:, k0:k0 + 4, :],
                    embN_r[:, k0:k0 + 4, d * 512:(d + 1) * 512],
                )
            st.setdefault(d, {})["eN"] = eN

        def ph2a(d):
            """A^T[g] = sum_n embN_chunk^T @ q: 4 banks over 16 n-chunks."""
            q_s, eN = st[d]["q_s"], st[d]["eN"]
            # pre-load the Sqrt ACT table off the critical path (Copy works
            # under any loaded set; the Exp table is pre-loaded in ph3)
            nc.scalar.activation(dmy[:, 1:2], dmy[:, 0:1], AF.Sqrt)
            ab = [acc() for _ in range(4)]
            for nk in range(NT):
                for g in range(4):
                    nc.tensor.matmul(
                        ab[g][:, :],
                        eN[:, nk * 512 + g * 128:nk * 512 + (g + 1) * 128],
                        q_s[:, nk * 512:(nk + 1) * 512],
                        start=(nk == 0), stop=(nk == NT - 1),
                    )
            AT = sb.tile([128, 4 * 512], bf16, tag="AT", bufs=2, name="AT")
            u = sb.tile([128, 4], f32, tag="u", name="u")
            for g in range(4):
                # u[sc] = sum_i A^T[sc, i] feeds the plane-mean path; split
                # engines so the L-matmuls can start after ~2 evictions
                if g < 2:
                    nc.scalar.activation(AT[:, g * 512:(g + 1) * 512],
                                         ab[g][:, :], AF.Copy,
                                         accum_out=u[:, g:g + 1])
                else:
                    nc.vector.tensor_copy(AT[:, g * 512:(g + 1) * 512],
                                          ab[g][:, :])
                    nc.vector.reduce_sum(u[:, g:g + 1], ab[g][:, :],
                                         axis=X.X)
            # B^T[(s,c), i] = sum_c' L[c',c] A^T[(s,c'), i]; sum B^2 = sum S^2
            # (K_hat = Wk Wk^T = L L^T, L from the host)
            bst = sb.tile([128, 4 * 6], f32, tag="bst", name="bst")
            for g in range(4):
                bb = bank()
                for z in range(2):
                    nc.tensor.matmul(
                        bb[z * 64:(z + 1) * 64, :],
                        L_s[z * 64:(z + 1) * 64, :],
                        AT[z * 64:(z + 1) * 64, g * 512:(g + 1) * 512],
                        start=True, stop=True,
                        tile_position=(z * 64, z * 64),
                    )
                nc.vector.bn_stats(bst[:, g * 6:(g + 1) * 6], bb[:, :])
            st[d]["AT"] = AT
            st[d]["u"] = u
            st[d]["bst"] = bst

        def m245(d):
            """Per s-pair g: scores matmuls + evictions, exp chunks,
            transpose batches, and G^T — fully pipelined since inv_sigma
            comes from the B-statistics (no dependence on the scores)."""
            AT = st[d]["AT"]
            invb = st[d]["invb"]
            kvs = (1 - d) * B
            scores_s = sb.tile([128, IT * J], bf16, tag="S", name="scores_s")
            attn_s = sb.tile([128, IT * J], bf16, tag="A", name="attn_s")
            # attnT layout: col = jk*512 + it*128 + ii (jk-major)
            attnT_s = sb.tile([128, IT * J], bf16, tag="T", name="attnT_s")
            aT3 = attnT_s.rearrange("p (j c) -> p j c", c=512)
            dparts = sb.tile([128, IT * 4], f32, tag="dp", name="dparts")
            GT = sb.tile([128, 4 * 512], bf16, tag="GT", name="GT")
            esc = st[d]["esc"]

            def transposes_gt(g):
                s0 = 2 * g
                for it in range(IT):
                    # 8 transposes (jk = 4*s0 .. 4*s0+7) batched per it
                    tp = ps.tile([128, 1024], bf16, tag="bank", bufs=4,
                                 name="tp")
                    for z in range(8):
                        jk = s0 * 4 + z
                        nc.tensor.transpose(
                            tp[:, z * 128:(z + 1) * 128],
                            attn_s[:, it * J + jk * 128:
                                   it * J + (jk + 1) * 128],
                            eye_s[:],
                        )
                    nc.vector.tensor_copy(
                        aT3[:, s0 * 4:s0 * 4 + 8, it * 128:(it + 1) * 128],
                        tp[:].rearrange("p (j c) -> p j c", c=128),
                    )
                # G^T for s0, s0+1: accumulate into bank g's halves
                gbank = acc()
                for z in range(2):
                    s = s0 + z
                    for ip in range(4):
                        jk = s * 4 + ip
                        nc.tensor.matmul(
                            gbank[z * 64:(z + 1) * 64, :],
                            WvT_s[:, ip * 64:(ip + 1) * 64],
                            attnT_s[:, jk * 512:(jk + 1) * 512],
                            start=(ip == 0), stop=(ip == 3),
                            tile_position=(0, z * 64),
                        )
                # DVE only: keeps the ACT queue a pure exp stream here
                nc.vector.tensor_copy(GT[:, g * 512:(g + 1) * 512],
                                      gbank[:, :])

            # pipeline: transposes/G^T run one exp-group behind so they
            # never wait on the ACT exp stream
            for g in range(4):
                s0 = 2 * g
                if len(esc) < 4:  # finish the embSC prefetches for ph5b
                    esc.append(_load_esc(kvs, len(esc)))
                # scores for s0, s0+1: single-shot K=64 row-tiled pairs
                for ic in range(IT):
                    p1, p2 = bank(), bank()
                    nc.tensor.matmul(
                        p1[:, :], AT[0:64, g * 512 + ic * 128:
                                     g * 512 + (ic + 1) * 128],
                        Wk_s[0:64, :], start=True, stop=True,
                        tile_position=(0, 0),
                    )
                    nc.tensor.matmul(
                        p2[:, :], AT[64:128, g * 512 + ic * 128:
                                     g * 512 + (ic + 1) * 128],
                        Wk_s[64:128, :], start=True, stop=True,
                        tile_position=(64, 0),
                    )
                    for z, pz in ((0, p1), (1, p2)):
                        s = s0 + z
                        dst = scores_s[:, ic * J + s * 512:
                                       ic * J + (s + 1) * 512]
                        if (ic * 2 + z) % 3 == 0:
                            nc.scalar.copy(dst, pz[:, :])
                        else:
                            nc.vector.tensor_copy(dst, pz[:, :])
                for it in range(IT):
                    nc.scalar.activation(
                        attn_s[:, it * J + s0 * 512:it * J + (s0 + 2) * 512],
                        scores_s[:, it * J + s0 * 512:
                                 it * J + (s0 + 2) * 512],
                        AF.Exp,
                        scale=invb[:],
                        accum_out=dparts[:, it * 4 + g:it * 4 + g + 1],
                    )
                if g > 0:
                    transposes_gt(g - 1)
            transposes_gt(3)

            denom = sb.tile([128, IT], f32, tag="denom")
            rden = sb.tile([128, IT], f32, tag="rden")
            for it in range(IT):
                nc.vector.reduce_sum(denom[:, it:it + 1],
                                     dparts[:, it * 4:(it + 1) * 4], axis=X.X)
            nc.vector.reciprocal(rden[:], denom[:])
            st[d]["GT"] = GT
            st[d]["rden"] = rden

        def _load_esc(kvs, g):
            """embSC tile g: samples 2g (rows 0-64) and 2g+1 (64-128)."""
            esc = sb.tile([128, N], bf16, tag="eS", bufs=4, name="esc")
            nc.sync.dma_start(
                esc[0:64, :],
                embT[:, (kvs + 2 * g) * N:(kvs + 2 * g + 1) * N])
            nc.sync.dma_start(
                esc[64:128, :],
                embT[:, (kvs + 2 * g + 1) * N:(kvs + 2 * g + 2) * N])
            return esc

        def ph3(d):
            """plane variance -> inv_sigma from the B-statistics (sum S^2 =
            sum B^2) and the u/wbar dot (sum S).  Exp bias is not needed
            (softmax shift-invariance) and |s*inv| < ~7 so no row-max."""
            u, bst = st[d]["u"], st[d]["bst"]
            kvs = (1 - d) * B
            esc = st[d].setdefault("esc", [])
            if d == 0:
                while len(esc) < 2:  # prefetch first embSC tiles for ph5b
                    esc.append(_load_esc(kvs, len(esc)))

            bsa = sb.tile([128, 2], f32, tag="cs", name="bsa")
            nc.vector.bn_aggr(bsa[:], bst[:])
            cstat = sb.tile([128, 2], f32, tag="cstat")
            # sumsqB_row = (varB + meanB^2) * 2048
            nc.vector.scalar_tensor_tensor(
                cstat[:, 1:2], bsa[:, 0:1], bsa[:, 0:1], bsa[:, 1:2],
                op0=ALU.mult, op1=ALU.add,
            )
            nc.vector.tensor_scalar_mul(cstat[:, 1:2], cstat[:, 1:2],
                                        float(4 * 512))
            # msum_row = sum_g u[r,g] * wbar[r]
            mp = sb.tile([128, 4], f32, tag="mp", name="mp")
            nc.vector.tensor_scalar(mp[:], u[:], wb_s[:], 1.0,
                                    op0=ALU.mult, op1=ALU.mult)
            nc.vector.reduce_sum(cstat[:, 0:1], mp[:], axis=X.X)
            # cross-partition totals via PE ones-vector matmul
            pst = bank()
            nc.tensor.matmul(pst[:1, :2], ones_t[:], cstat[:],
                             start=True, stop=True)
            tstat = sb.tile([1, 2], f32, tag="tstat")
            nc.vector.tensor_copy(tstat[:], pst[:1, :2])
            mean = sb.tile([1, 1], f32, tag="mean")
            ex2 = sb.tile([1, 1], f32, tag="ex2")
            INVM = 1.0 / float(CH * J)
            nc.vector.tensor_scalar_mul(mean[:], tstat[:, 0:1], INVM)
            nc.vector.tensor_scalar_mul(ex2[:], tstat[:, 1:2], INVM)
            negvar = sb.tile([1, 1], f32, tag="negvar")
            nc.vector.scalar_tensor_tensor(
                negvar[:], mean[:], mean[:], ex2[:],
                op0=ALU.mult, op1=ALU.subtract,
            )
            sig = sb.tile([1, 1], f32, tag="sig")
            # sqrt(-negvar + eps) = sqrt(var + eps); table pre-loaded
            nc.scalar.activation(sig[:], negvar[:], AF.Sqrt,
                                 bias=eps_t[:], scale=-1.0)
            # pre-load the Exp table while DVE finishes the chain
            nc.scalar.activation(dmy[:, 1:2], dmy[:, 0:1], AF.Exp)
            inv = sb.tile([1, 1], f32, tag="inv")
            nc.vector.reciprocal(inv[:], sig[:])
            invb = sb.tile([128, 1], f32, tag="invb")
            nc.gpsimd.partition_broadcast(invb[:], inv[:])
            st[d]["invb"] = invb

        def ph5b(d):
            """ctxT[ic, nt] = sum_g GT_g(ic)^T @ embSC_g; /denom folded into
            the eviction."""
            GT, rden, esc = st[d]["GT"], st[d]["rden"], st[d]["esc"]
            ctxT_s = sb.tile([128, IT * 4 * 512], bf16, tag="A",
                             name="ctxT_s")
            for ic in range(IT):
                for nt in range(4):
                    cp = acc()
                    for g in range(4):
                        nc.tensor.matmul(
                            cp[:, :],
                            GT[:, g * 512 + ic * 128:g * 512 + (ic + 1) * 128],
                            esc[g][:, nt * 512:(nt + 1) * 512],
                            start=(g == 0), stop=(g == 3),
                        )
                    dst = ctxT_s[:, (ic * 4 + nt) * 512:
                                 (ic * 4 + nt + 1) * 512]
                    if nt & 1:
                        nc.vector.tensor_scalar_mul(dst, cp[:, :],
                                                    rden[:, ic:ic + 1])
                    else:
                        nc.scalar.activation(dst, cp[:, :], AF.Copy,
                                             scale=rden[:, ic:ic + 1])
            st[d]["ctxT_s"] = ctxT_s

        def ph6(d):
            """output projection + chunked DMA."""
            ctxT_s = st[d]["ctxT_s"]
            out_s = sb.tile([128, 16 * 64], f32, tag="out_s", name="out_s")
            odst = out_d[d].rearrange("(g p) c -> p g c", p=128)
            osrc = out_s[:].rearrange("p (g c) -> p g c", g=NT)
            for g in range(NT):
                nt, sub = g >> 2, g & 3
                op = acc()
                for it in range(IT):
                    nc.tensor.matmul(
                        op[:, :64],
                        ctxT_s[:, (it * 4 + nt) * 512 + sub * 128:
                               (it * 4 + nt) * 512 + (sub + 1) * 128],
                        Wo_s[:, it * 64:(it + 1) * 64],
                        start=(it == 0), stop=(it == IT - 1),
                    )
                if g & 1:
                    nc.scalar.copy(out_s[:, g * 64:(g + 1) * 64], op[:, :64])
                else:
                    nc.vector.tensor_copy(out_s[:, g * 64:(g + 1) * 64],
                                          op[:, :64])
                if g & 3 == 3:
                    nc.sync.dma_start(odst[:, g - 3:g + 1, :],
                                      osrc[:, g - 3:g + 1, :])

        import contextlib
        loop_cm = (tc.For_i(0, REPEAT, 1) if REPEAT > 1
                   else contextlib.nullcontext())
        with loop_cm:
            ph1(0)
            prefetch_eN(0)
            # remaining persistent weights, behind ph1's inputs on the queue
            Wk_s = sb.tile([128, 512], bf16)
            nc.sync.dma_start(Wk_s[0:64, :], t["Wk"])
            nc.sync.dma_start(Wk_s[64:128, :], t["Wk"])
            WvT_s = sb.tile([128, IT * 64], bf16)
            wvt_r = t["WvT"].rearrange("(k p) c -> k p c", p=128)
            for i in range(IT):
                nc.sync.dma_start(WvT_s[:, i * 64:(i + 1) * 64], wvt_r[i])
            Wo_s = sb.tile([128, IT * 64], bf16)
            wo_r = t["Wo"].rearrange("(t p) c -> t p c", p=128)
            for i in range(IT):
                nc.sync.dma_start(Wo_s[:, i * 64:(i + 1) * 64], wo_r[i])
            eye_s = sb.tile([128, 128], bf16)
            nc.sync.dma_start(eye_s[:], t["eye"])
            L_s = sb.tile([128, 64], bf16)
            nc.sync.dma_start(L_s[:], t["L"])
            wb_s = sb.tile([128, 1], f32)
            nc.sync.dma_start(wb_s[:], t["wb"])

            ph2a(0)
            ph3(0)     # stats need only ph2a outputs now
            prefetch_eN(1)
            m245(0)
            ph1(1)     # ACT/DVE drain m245's tail during these matmuls
            ph2a(1)
            ph5b(0)
            ph3(1)     # d1 stats chain runs under ph6(0)
            ph6(0)
            esc1 = st[1].setdefault("esc", [])
            while len(esc1) < 4:   # d1 embSC after d0's are consumed
                esc1.append(_load_esc(0 * B, len(esc1)))
            m245(1)
            ph5b(1)
            ph6(1)


def _build():
    key = ("nc", REPEAT)
    if key in _CACHE:
        return _CACHE[key]
    from concourse import bass, bacc, tile, mybir

    f32 = mybir.dt.float32
    bf16 = mybir.dt.bfloat16

    nc = bacc.Bacc("TRN2", target_bir_lowering=False, debug=False)
    t = {
        "embT": nc.dram_tensor("embT", [C, 2 * B * N], bf16,
                               kind="ExternalInput").ap(),
        "embTq": nc.dram_tensor("embTq", [C, 2 * N], bf16,
                                kind="ExternalInput").ap(),
        "embN": nc.dram_tensor("embN", [N, 2 * B * C], bf16,
                               kind="ExternalInput").ap(),
        "Wq": nc.dram_tensor("Wq", [C, CH], bf16, kind="ExternalInput").ap(),
        "Wk": nc.dram_tensor("Wk", [C, CH], bf16, kind="ExternalInput").ap(),
        "WvT": nc.dram_tensor("WvT", [CH, C], bf16,
                              kind="ExternalInput").ap(),
        "Wo": nc.dram_tensor("Wo", [CH, C], bf16, kind="ExternalInput").ap(),
        "eye": nc.dram_tensor("eye", [128, 128], bf16,
                              kind="ExternalInput").ap(),
        "L": nc.dram_tensor("L", [128, 64], bf16,
                            kind="ExternalInput").ap(),
        "wb": nc.dram_tensor("wb", [128, 1], f32,
                             kind="ExternalInput").ap(),
        "out": nc.dram_tensor("out", [2, N, C], f32,
                              kind="ExternalOutput").ap(),
    }
    with tile.TileContext(nc) as tc:
        _emit(nc, tc, bass, mybir, t)
    nc.compile()
    _CACHE[key] = nc
    return nc


def kernel(emb, Wq, Wk, Wv, Wo):
    from concourse.bass_utils import run_bass_kernel_spmd

    bf = ml_dtypes.bfloat16
    emb = np.asarray(emb, dtype=np.float32)
    Wq = np.ascontiguousarray(np.asarray(Wq, dtype=np.float32)).astype(bf)
    Wk = np.ascontiguousarray(np.asarray(Wk, dtype=np.float32)).astype(bf)
    WvT = np.ascontiguousarray(
        np.asarray(Wv, dtype=np.float32).T).astype(bf)
    Wo = np.ascontiguousarray(np.asarray(Wo, dtype=np.float32)).astype(bf)

    nc = _build()

    embT = np.ascontiguousarray(
        emb.transpose(2, 0, 1).reshape(C, 2 * B * N)).astype(bf)
    # embN[n, d*512 + s*64+c] = emb[kv_half(d) sample s, n, c]
    embN = np.empty((N, 2 * B * C), dtype=np.float32)
    embN[:, 0:B * C] = emb[B:].transpose(1, 0, 2).reshape(N, B * C)
    embN[:, B * C:] = emb[:B].transpose(1, 0, 2).reshape(N, B * C)
    embN = np.ascontiguousarray(embN).astype(bf)
    eye = np.eye(128, dtype=bf)
    # Cholesky factor of Wk Wk^T (for sum S^2 = ||A L||_F^2) and Wk row
    # sums (for sum S), both duplicated into the partition halves
    Wk32 = Wk.astype(np.float32)
    Khat = Wk32 @ Wk32.T
    Lc = np.linalg.cholesky(Khat + 1e-10 * np.eye(C))
    Ldup = np.ascontiguousarray(np.vstack([Lc, Lc])).astype(bf)
    wbar = Wk32.sum(1)
    wbdup = np.ascontiguousarray(
        np.concatenate([wbar, wbar])[:, None].astype(np.float32))

    in_maps = []
    for c in range(B):
        embTq = np.ascontiguousarray(
            np.concatenate([embT[:, c * N:(c + 1) * N],
                            embT[:, (B + c) * N:(B + c + 1) * N]], axis=1))
        in_maps.append({
            "embT": embT, "embTq": embTq, "embN": embN,
            "Wq": Wq, "Wk": Wk, "WvT": WvT, "Wo": Wo, "eye": eye,
            "L": Ldup, "wb": wbdup,
        })

    r = run_bass_kernel_spmd(nc, in_maps, list(range(B)))
    res = r.results

    out = np.empty((2 * B, N, C), dtype=np.float32)
    for c in range(B):
        out[c] = res[c]["out"][0]
        out[B + c] = res[c]["out"][1]
    return out


# revision 22
# speedup vs baseline: 1.3041x; 1.0761x over previous
"""Trainium2 Bass kernel for nn_Cross_Attention_Global (sparse_attention).

Key algebraic identity: the [512, 4096] score block per (query sample b,
kv sample s) is a product of rank-64 factors, so contract over the embed
dim c=64 FIRST:

  scores_{b,s} = q_b^T (emb_s Wk) = (q_b^T emb_s) Wk = A_s Wk
  ctx_b = sum_s emb_s (Wv attn_{b,s}^T)^T = sum_s emb_s G_s^T

This cuts the big matmuls ~6x vs the direct form (1.34 GFLOP instead of
8.6 GFLOP per plane and per path).

Math per direction d (query sample b owned by this core, kv half KV):
  q_b = Q[b] @ Wq                                   [2048, 512]
  A^T[s*64+c, i] = sum_n KV[s,n,c] q_b[n,i]         [512, 512]
  scores[i, s*512+i'] = sum_c A^T[s*64+c, i] Wk[c,i']
  InstanceNorm over plane + softmax over j: the per-plane mean cancels in
  softmax, so only inv_sigma = rsqrt(var+eps) matters; scores*inv_sigma is
  bounded (|z| < ~7) so exp needs no row-max subtraction either.
  attn = exp(scores * inv_sigma); denom = rowsum (via exp accum_out)
  G^T[s*64+c, i] = sum_i' Wv[c,i'] attnT[(s,i'), i]  [512, 512]
  ctxT[i, n] = sum_{s,c} G^T[s*64+c, i] KV[s,n,c]    (/denom at eviction)
  out_b = ctxT^T @ Wo                                [2048, 64]

Sharding: core c handles query sample c of direction 0 (q=emb_l, kv=emb_u)
and query sample c of direction 1 (q=emb_u, kv=emb_l).  No collectives.

All matmul operands bf16 (fp32 PSUM accumulation); emb/weights converted
to bf16 on the host.  End-to-end rel-err ~6e-3 vs the 2e-2 gate.

Per-core structure, per direction:
  ph1:  q projection (row-tiled pairs of K=64 matmuls)
  ph2a: A^T, 4 PSUM banks (g = s-pairs) accumulating over 16 n-chunks
  ph2b: scores: single-shot K=64 row-tiled pair matmuls (s even/odd in
        AT partition halves); evictions split ACT/DVE; bn_stats on the
        bf16 scores for the plane variance
  ph3:  bn_aggr -> inv_sigma (cross-partition totals via PE ones matmul);
        ACT table pre-loads hide the Sqrt/Exp table switches
  ph4+5a (per s-pair): exp chunks [128,1024] (+denoms via accum_out),
        per-it transpose batches -> attnT, then G^T matmuls per s
  ph5b: ctxT = sum_g GT_g^T @ embSC_g per (i-chunk, n-block); 1/denom
        folded into the eviction
  ph6:  Wo projection, chunked output DMA
Direction 1's ph1+ph2a fill direction 0's scores-eviction/stats window;
direction 1's ph2b fills the gap before ph5b(d0).
"""

import sys

sys.path.insert(0, "/opt/trn_rl_repo")

import numpy as np
import ml_dtypes

_CACHE = {}
REPEAT = 1   # timing knob: execute the whole computation REPEAT times

B = 8          # samples per half-batch
N = 2048       # sequence length
C = 64         # embed dim
CH = 512       # head dim total (q/k/v channels)
J = B * CH     # 4096, kv concat width
NT = N // 128  # 16 n-tiles
IT = CH // 128  # 4 i-tiles
EPS = 1e-5


def _emit(nc, tc, bass, mybir, t):
    f32 = mybir.dt.float32
    bf16 = mybir.dt.bfloat16
    X = mybir.AxisListType
    AF = mybir.ActivationFunctionType
    ALU = mybir.AluOpType

    embT = t["embT"]    # [64, 16*2048] bf16, col = sglobal*2048+n
    embTq = t["embTq"]  # [64, 2*2048] bf16, per-core q samples (dir0, dir1)
    embN = t["embN"]    # [2048, 2*512] bf16, col = d*512 + s*64+c
    out_d = t["out"]    # [2, 2048, 64] f32

    embN_r = embN.rearrange("(k p) s -> p k s", p=128)  # [128, 16, 1024]

    with (
        tc.tile_pool(name="sb", bufs=1) as sb,
        tc.tile_pool(name="ps", bufs=8, space=bass.MemorySpace.PSUM) as ps,
    ):
        def acc():
            return ps.tile([128, 512], f32, tag="acc", bufs=4, name="acc")

        def bankw():
            # two-bank PSUM tile: row-tiled matmul pairs write the halves,
            # one 1024-wide op evicts (or exps) both
            return ps.tile([128, 1024], f32, tag="bankw", bufs=2,
                           name="bankw")

        # --- persistent weights (Wq/Wk duplicated into both partition
        # halves so row-tiled pair matmuls can source at base_partition 64)
        Wq_s = sb.tile([128, 512], bf16)
        nc.sync.dma_start(Wq_s[0:64, :], t["Wq"])
        nc.sync.dma_start(Wq_s[64:128, :], t["Wq"])

        eps_t = sb.tile([1, 1], f32, tag="eps_t")
        nc.gpsimd.memset(eps_t[:], float(EPS))
        ones_t = sb.tile([128, 1], f32, tag="ones_t")
        nc.gpsimd.memset(ones_t[:], 1.0)
        dmy = sb.tile([1, 2], f32, tag="dmy")
        nc.gpsimd.memset(dmy[:], 1.0)

        st = {}  # per-dir tile state

        def ph1(d):
            """q projection for dir d + embN prefetch for dir d."""
            qoff = d * N
            q_s = sb.tile([128, NT * 512], bf16, tag="q_s", name="q_s")
            eqs = sb.tile([128, N], bf16, tag="eqs", name="eqs")
            nc.sync.dma_start(eqs[0:64, :], embTq[:, qoff:qoff + N])
            nc.sync.dma_start(eqs[64:128, :], embTq[:, qoff:qoff + N])
            for nk in range(0, NT, 2):
                qw = bankw()
                nc.tensor.matmul(
                    qw[:, 0:512], eqs[0:64, nk * 128:(nk + 1) * 128],
                    Wq_s[0:64, :], start=True, stop=True,
                    tile_position=(0, 0),
                )
                nc.tensor.matmul(
                    qw[:, 512:1024], eqs[64:128, (nk + 1) * 128:(nk + 2) * 128],
                    Wq_s[64:128, :], start=True, stop=True,
                    tile_position=(64, 0),
                )
                if (nk >> 1) & 1:
                    nc.scalar.copy(q_s[:, nk * 512:(nk + 2) * 512], qw[:, :])
                else:
                    nc.vector.tensor_copy(q_s[:, nk * 512:(nk + 2) * 512],
                                          qw[:, :])
            st.setdefault(d, {})["q_s"] = q_s

        def prefetch_eN(d):
            """n-major kv embedding for ph2a(d): 2 MB DMA, issued early."""
            eN = sb.tile([128, NT * 512], bf16, tag="eN", bufs=2, name="eN")
            eN3 = eN[:].rearrange("p (k s) -> p k s", s=512)
            for k0 in range(0, NT, 4):   # chunked: ph2a starts on chunk 0
                nc.sync.dma_start(
                    eN3[:, k0:k0 + 4, :],
                    embN_r[:, k0:k0 + 4, d * 512:(d + 1) * 512],
                )
            st.setdefault(d, {})["eN"] = eN

        def ph2a(d):
            """A^T[g] = sum_n embN_chunk^T @ q: 4 banks over 16 n-chunks."""
            q_s, eN = st[d]["q_s"], st[d]["eN"]
            # pre-load the Sqrt ACT table off the critical path (Copy works
            # under any loaded set; the Exp table is pre-loaded in ph3)
            nc.scalar.activation(dmy[:, 1:2], dmy[:, 0:1], AF.Sqrt)
            ab = [acc() for _ in range(4)]
            for nk in range(NT):
                for g in range(4):
                    nc.tensor.matmul(
                        ab[g][:, :],
                        eN[:, nk * 512 + g * 128:nk * 512 + (g + 1) * 128],
                        q_s[:, nk * 512:(nk + 1) * 512],
                        start=(nk == 0), stop=(nk == NT - 1),
                    )
            AT = sb.tile([128, 4 * 512], bf16, tag="AT", bufs=2, name="AT")
            u = sb.tile([128, 4], f32, tag="u", name="u")
            for g in range(4):
                # u[sc] = sum_i A^T[sc, i] feeds the plane-mean path; split
                # engines so the L-matmuls can start after ~2 evictions
                if g < 2:
                    nc.scalar.activation(AT[:, g * 512:(g + 1) * 512],
                                         ab[g][:, :], AF.Copy,
                                         accum_out=u[:, g:g + 1])
                else:
                    nc.vector.tensor_copy(AT[:, g * 512:(g + 1) * 512],
                                          ab[g][:, :])
                    nc.vector.reduce_sum(u[:, g:g + 1], ab[g][:, :],
                                         axis=X.X)
            # B^T[(s,c), i] = sum_c' L[c',c] A^T[(s,c'), i]; sum B^2 = sum S^2
            # (K_hat = Wk Wk^T = L L^T, L from the host)
            bst = sb.tile([128, 4 * 6], f32, tag="bst", name="bst")
            for gp in range(2):
                bb = bankw()
                for gz in range(2):
                    g = gp * 2 + gz
                    for z in range(2):
                        nc.tensor.matmul(
                            bb[z * 64:(z + 1) * 64,
                               gz * 512:(gz + 1) * 512],
                            L_s[z * 64:(z + 1) * 64, :],
                            AT[z * 64:(z + 1) * 64, g * 512:(g + 1) * 512],
                            start=True, stop=True,
                            tile_position=(z * 64, z * 64),
                        )
                for gz in range(2):
                    g = gp * 2 + gz
                    nc.vector.bn_stats(bst[:, g * 6:(g + 1) * 6],
                                       bb[:, gz * 512:(gz + 1) * 512])
            st[d]["AT"] = AT
            st[d]["u"] = u
            st[d]["bst"] = bst

        def m245(d):
            """Per s-pair g: scores matmuls + evictions, exp chunks,
            transpose batches, and G^T — fully pipelined since inv_sigma
            comes from the B-statistics (no dependence on the scores)."""
            AT = st[d]["AT"]
            invb = st[d]["invb"]
            kvs = (1 - d) * B
            attn_s = sb.tile([128, IT * J], bf16, tag="A", name="attn_s")
            # attnT layout: col = jk*512 + it*128 + ii (jk-major)
            attnT_s = sb.tile([128, IT * J], bf16, tag="T", name="attnT_s")
            aT3 = attnT_s.rearrange("p (j c) -> p j c", c=512)
            dparts = sb.tile([128, IT * 4], f32, tag="dp", name="dparts")
            GT = sb.tile([128, 4 * 512], bf16, tag="GT", name="GT")
            esc = st[d]["esc"]

            def transposes_gt(g, its):
                s0 = 2 * g
                for it in its:
                    # 8 transposes (jk = 4*s0 .. 4*s0+7) batched per it
                    tp = ps.tile([128, 1024], bf16, tag="acc", bufs=4,
                                 name="tp")
                    for z in range(8):
                        jk = s0 * 4 + z
                        nc.tensor.transpose(
                            tp[:, z * 128:(z + 1) * 128],
                            attn_s[:, it * J + jk * 128:
                                   it * J + (jk + 1) * 128],
                            eye_s[:],
                        )
                    nc.vector.tensor_copy(
                        aT3[:, s0 * 4:s0 * 4 + 8, it * 128:(it + 1) * 128],
                        tp[:].rearrange("p (j c) -> p j c", c=128),
                    )

            def gt_mms(g):
                # G^T for 2g, 2g+1: accumulate into bank g's halves
                gbank = acc()
                for z in range(2):
                    s = 2 * g + z
                    for ip in range(4):
                        jk = s * 4 + ip
                        nc.tensor.matmul(
                            gbank[z * 64:(z + 1) * 64, :],
                            WvT_s[:, ip * 64:(ip + 1) * 64],
                            attnT_s[:, jk * 512:(jk + 1) * 512],
                            start=(ip == 0), stop=(ip == 3),
                            tile_position=(0, z * 64),
                        )
                # DVE only: keeps the ACT queue a pure exp stream here
                nc.vector.tensor_copy(GT[:, g * 512:(g + 1) * 512],
                                      gbank[:, :])

            def sc_exp(g, ic):
                """scores pair (ic; s=2g,2g+1) into a two-bank PSUM tile,
                exp'd straight out of PSUM (no SBUF scores at all)."""
                s0 = 2 * g
                bw = bankw()
                nc.tensor.matmul(
                    bw[:, 0:512], AT[0:64, g * 512 + ic * 128:
                                     g * 512 + (ic + 1) * 128],
                    Wk_s[0:64, :], start=True, stop=True,
                    tile_position=(0, 0),
                )
                nc.tensor.matmul(
                    bw[:, 512:1024], AT[64:128, g * 512 + ic * 128:
                                        g * 512 + (ic + 1) * 128],
                    Wk_s[64:128, :], start=True, stop=True,
                    tile_position=(64, 0),
                )
                nc.scalar.activation(
                    attn_s[:, ic * J + s0 * 512:ic * J + (s0 + 2) * 512],
                    bw[:, :], AF.Exp, scale=invb[:],
                    accum_out=dparts[:, ic * 4 + g:ic * 4 + g + 1],
                )

            # pipeline: transposes/G^T run one exp-group behind, split
            # around the mid-g score matmuls to absorb bankw rotation waits
            for g in range(4):
                if len(esc) < 4:  # finish the embSC prefetches for ph5b
                    esc.append(_load_esc(kvs, len(esc)))
                sc_exp(g, 0)
                sc_exp(g, 1)
                if g > 0:
                    transposes_gt(g - 1, (0, 1))
                sc_exp(g, 2)
                sc_exp(g, 3)
                if g > 0:
                    transposes_gt(g - 1, (2, 3))
                    gt_mms(g - 1)
            transposes_gt(3, (0, 1, 2, 3))
            gt_mms(3)

            denom = sb.tile([128, IT], f32, tag="denom")
            rden = sb.tile([128, IT], f32, tag="rden")
            for it in range(IT):
                nc.vector.reduce_sum(denom[:, it:it + 1],
                                     dparts[:, it * 4:(it + 1) * 4], axis=X.X)
            nc.vector.reciprocal(rden[:], denom[:])
            st[d]["GT"] = GT
            st[d]["rden"] = rden

        def _load_esc(kvs, g):
            """embSC tile g: samples 2g (rows 0-64) and 2g+1 (64-128)."""
            esc = sb.tile([128, N], bf16, tag="eS", bufs=4, name="esc")
            nc.sync.dma_start(
                esc[0:64, :],
                embT[:, (kvs + 2 * g) * N:(kvs + 2 * g + 1) * N])
            nc.sync.dma_start(
                esc[64:128, :],
                embT[:, (kvs + 2 * g + 1) * N:(kvs + 2 * g + 2) * N])
            return esc

        def ph3(d):
            """plane variance -> inv_sigma from the B-statistics (sum S^2 =
            sum B^2) and the u/wbar dot (sum S).  Exp bias is not needed
            (softmax shift-invariance) and |s*inv| < ~7 so no row-max."""
            u, bst = st[d]["u"], st[d]["bst"]
            kvs = (1 - d) * B
            esc = st[d].setdefault("esc", [])
            if d == 0:
                while len(esc) < 2:  # prefetch first embSC tiles for ph5b
                    esc.append(_load_esc(kvs, len(esc)))

            bsa = sb.tile([128, 2], f32, tag="cs", name="bsa")
            nc.vector.bn_aggr(bsa[:], bst[:])
            cstat = sb.tile([128, 2], f32, tag="cstat")
            # sumsqB_row = (varB + meanB^2) * 2048
            nc.vector.scalar_tensor_tensor(
                cstat[:, 1:2], bsa[:, 0:1], bsa[:, 0:1], bsa[:, 1:2],
                op0=ALU.mult, op1=ALU.add,
            )
            nc.vector.tensor_scalar_mul(cstat[:, 1:2], cstat[:, 1:2],
                                        float(4 * 512))
            # msum_row = sum_g u[r,g] * wbar[r]
            mp = sb.tile([128, 4], f32, tag="mp", name="mp")
            nc.vector.tensor_scalar(mp[:], u[:], wb_s[:], 1.0,
                                    op0=ALU.mult, op1=ALU.mult)
            nc.vector.reduce_sum(cstat[:, 0:1], mp[:], axis=X.X)
            # cross-partition totals via PE ones-vector matmul
            pst = bankw()
            nc.tensor.matmul(pst[:1, :2], ones_t[:], cstat[:],
                             start=True, stop=True)
            tstat = sb.tile([1, 2], f32, tag="tstat")
            nc.vector.tensor_copy(tstat[:], pst[:1, :2])
            mean = sb.tile([1, 1], f32, tag="mean")
            ex2 = sb.tile([1, 1], f32, tag="ex2")
            INVM = 1.0 / float(CH * J)
            nc.vector.tensor_scalar_mul(mean[:], tstat[:, 0:1], INVM)
            nc.vector.tensor_scalar_mul(ex2[:], tstat[:, 1:2], INVM)
            negvar = sb.tile([1, 1], f32, tag="negvar")
            nc.vector.scalar_tensor_tensor(
                negvar[:], mean[:], mean[:], ex2[:],
                op0=ALU.mult, op1=ALU.subtract,
            )
            sig = sb.tile([1, 1], f32, tag="sig")
            # sqrt(-negvar + eps) = sqrt(var + eps); table pre-loaded
            nc.scalar.activation(sig[:], negvar[:], AF.Sqrt,
                                 bias=eps_t[:], scale=-1.0)
            # pre-load the Exp table while DVE finishes the chain
            nc.scalar.activation(dmy[:, 1:2], dmy[:, 0:1], AF.Exp)
            inv = sb.tile([1, 1], f32, tag="inv")
            nc.vector.reciprocal(inv[:], sig[:])
            invb = sb.tile([128, 1], f32, tag="invb")
            nc.gpsimd.partition_broadcast(invb[:], inv[:])
            st[d]["invb"] = invb

        def ph5b(d):
            """ctxT[ic, nt] = sum_g GT_g(ic)^T @ embSC_g; /denom folded into
            the eviction."""
            GT, rden, esc = st[d]["GT"], st[d]["rden"], st[d]["esc"]
            ctxT_s = sb.tile([128, IT * 4 * 512], bf16, tag="A",
                             name="ctxT_s")
            for ic in range(IT):
                for nt in range(4):
                    cp = acc()
                    for g in range(4):
                        nc.tensor.matmul(
                            cp[:, :],
                            GT[:, g * 512 + ic * 128:g * 512 + (ic + 1) * 128],
                            esc[g][:, nt * 512:(nt + 1) * 512],
                            start=(g == 0), stop=(g == 3),
                        )
                    dst = ctxT_s[:, (ic * 4 + nt) * 512:
                                 (ic * 4 + nt + 1) * 512]
                    if nt & 1:
                        nc.vector.tensor_scalar_mul(dst, cp[:, :],
                                                    rden[:, ic:ic + 1])
                    else:
                        nc.scalar.activation(dst, cp[:, :], AF.Copy,
                                             scale=rden[:, ic:ic + 1])
            st[d]["ctxT_s"] = ctxT_s

        def ph6(d):
            """output projection + chunked DMA."""
            ctxT_s = st[d]["ctxT_s"]
            out_s = sb.tile([128, 16 * 64], f32, tag="out_s", name="out_s")
            odst = out_d[d].rearrange("(g p) c -> p g c", p=128)
            osrc = out_s[:].rearrange("p (g c) -> p g c", g=NT)
            for g in range(NT):
                nt, sub = g >> 2, g & 3
                op = acc()
                for it in range(IT):
                    nc.tensor.matmul(
                        op[:, :64],
                        ctxT_s[:, (it * 4 + nt) * 512 + sub * 128:
                               (it * 4 + nt) * 512 + (sub + 1) * 128],
                        Wo_s[:, it * 64:(it + 1) * 64],
                        start=(it == 0), stop=(it == IT - 1),
                    )
                if g & 1:
                    nc.scalar.copy(out_s[:, g * 64:(g + 1) * 64], op[:, :64])
                else:
                    nc.vector.tensor_copy(out_s[:, g * 64:(g + 1) * 64],
                                          op[:, :64])
                if g & 3 == 3:
                    nc.sync.dma_start(odst[:, g - 3:g + 1, :],
                                      osrc[:, g - 3:g + 1, :])

        import contextlib
        loop_cm = (tc.For_i(0, REPEAT, 1) if REPEAT > 1
                   else contextlib.nullcontext())
        with loop_cm:
            ph1(0)
            prefetch_eN(0)
            # remaining persistent weights, behind ph1's inputs on the queue
            Wk_s = sb.tile([128, 512], bf16)
            nc.sync.dma_start(Wk_s[0:64, :], t["Wk"])
            nc.sync.dma_start(Wk_s[64:128, :], t["Wk"])
            WvT_s = sb.tile([128, IT * 64], bf16)
            wvt_r = t["WvT"].rearrange("(k p) c -> k p c", p=128)
            for i in range(IT):
                nc.sync.dma_start(WvT_s[:, i * 64:(i + 1) * 64], wvt_r[i])
            Wo_s = sb.tile([128, IT * 64], bf16)
            wo_r = t["Wo"].rearrange("(t p) c -> t p c", p=128)
            for i in range(IT):
                nc.sync.dma_start(Wo_s[:, i * 64:(i + 1) * 64], wo_r[i])
            eye_s = sb.tile([128, 128], bf16)
            nc.sync.dma_start(eye_s[:], t["eye"])
            L_s = sb.tile([128, 64], bf16)
            nc.sync.dma_start(L_s[:], t["L"])
            wb_s = sb.tile([128, 1], f32)
            nc.sync.dma_start(wb_s[:], t["wb"])

            ph2a(0)
            ph3(0)     # stats need only ph2a outputs now
            prefetch_eN(1)
            m245(0)
            ph1(1)     # ACT/DVE drain m245's tail during these matmuls
            ph2a(1)
            ph5b(0)
            ph3(1)     # d1 stats chain runs under ph6(0)
            ph6(0)
            esc1 = st[1].setdefault("esc", [])
            while len(esc1) < 4:   # d1 embSC after d0's are consumed
                esc1.append(_load_esc(0 * B, len(esc1)))
            m245(1)
            ph5b(1)
            ph6(1)


def _build():
    key = ("nc", REPEAT)
    if key in _CACHE:
        return _CACHE[key]
    from concourse import bass, bacc, tile, mybir

    f32 = mybir.dt.float32
    bf16 = mybir.dt.bfloat16

    nc = bacc.Bacc("TRN2", target_bir_lowering=False, debug=False)
    t = {
        "embT": nc.dram_tensor("embT", [C, 2 * B * N], bf16,
                               kind="ExternalInput").ap(),
        "embTq": nc.dram_tensor("embTq", [C, 2 * N], bf16,
                                kind="ExternalInput").ap(),
        "embN": nc.dram_tensor("embN", [N, 2 * B * C], bf16,
                               kind="ExternalInput").ap(),
        "Wq": nc.dram_tensor("Wq", [C, CH], bf16, kind="ExternalInput").ap(),
        "Wk": nc.dram_tensor("Wk", [C, CH], bf16, kind="ExternalInput").ap(),
        "WvT": nc.dram_tensor("WvT", [CH, C], bf16,
                              kind="ExternalInput").ap(),
        "Wo": nc.dram_tensor("Wo", [CH, C], bf16, kind="ExternalInput").ap(),
        "eye": nc.dram_tensor("eye", [128, 128], bf16,
                              kind="ExternalInput").ap(),
        "L": nc.dram_tensor("L", [128, 64], bf16,
                            kind="ExternalInput").ap(),
        "wb": nc.dram_tensor("wb", [128, 1], f32,
                             kind="ExternalInput").ap(),
        "out": nc.dram_tensor("out", [2, N, C], f32,
                              kind="ExternalOutput").ap(),
    }
    with tile.TileContext(nc) as tc:
        _emit(nc, tc, bass, mybir, t)
    nc.compile()
    _CACHE[key] = nc
    return nc


def kernel(emb, Wq, Wk, Wv, Wo):
    from concourse.bass_utils import run_bass_kernel_spmd

    bf = ml_dtypes.bfloat16
    emb = np.asarray(emb, dtype=np.float32)
    Wq = np.ascontiguousarray(np.asarray(Wq, dtype=np.float32)).astype(bf)
    Wk = np.ascontiguousarray(np.asarray(Wk, dtype=np.float32)).astype(bf)
    WvT = np.ascontiguousarray(
        np.asarray(Wv, dtype=np.float32).T).astype(bf)
    Wo = np.ascontiguousarray(np.asarray(Wo, dtype=np.float32)).astype(bf)

    nc = _build()

    embT = np.ascontiguousarray(
        emb.transpose(2, 0, 1).reshape(C, 2 * B * N)).astype(bf)
    # embN[n, d*512 + s*64+c] = emb[kv_half(d) sample s, n, c]
    embN = np.empty((N, 2 * B * C), dtype=np.float32)
    embN[:, 0:B * C] = emb[B:].transpose(1, 0, 2).reshape(N, B * C)
    embN[:, B * C:] = emb[:B].transpose(1, 0, 2).reshape(N, B * C)
    embN = np.ascontiguousarray(embN).astype(bf)
    eye = np.eye(128, dtype=bf)
    # Cholesky factor of Wk Wk^T (for sum S^2 = ||A L||_F^2) and Wk row
    # sums (for sum S), both duplicated into the partition halves
    Wk32 = Wk.astype(np.float32)
    Khat = Wk32 @ Wk32.T
    Lc = np.linalg.cholesky(Khat + 1e-10 * np.eye(C))
    Ldup = np.ascontiguousarray(np.vstack([Lc, Lc])).astype(bf)
    wbar = Wk32.sum(1)
    wbdup = np.ascontiguousarray(
        np.concatenate([wbar, wbar])[:, None].astype(np.float32))

    in_maps = []
    for c in range(B):
        embTq = np.ascontiguousarray(
            np.concatenate([embT[:, c * N:(c + 1) * N],
                            embT[:, (B + c) * N:(B + c + 1) * N]], axis=1))
        in_maps.append({
            "embT": embT, "embTq": embTq, "embN": embN,
            "Wq": Wq, "Wk": Wk, "WvT": WvT, "Wo": Wo, "eye": eye,
            "L": Ldup, "wb": wbdup,
        })

    r = run_bass_kernel_spmd(nc, in_maps, list(range(B)))
    res = r.results

    out = np.empty((2 * B, N, C), dtype=np.float32)
    for c in range(B):
        out[c] = res[c]["out"][0]
        out[B + c] = res[c]["out"][1]
    return out


# revision 23
# speedup vs baseline: 1.4379x; 1.1026x over previous
"""Trainium2 Bass kernel for nn_Cross_Attention_Global (sparse_attention).

Key algebraic identity: the [512, 4096] score block per (query sample b,
kv sample s) is a product of rank-64 factors, so contract over the embed
dim c=64 FIRST:

  scores_{b,s} = q_b^T (emb_s Wk) = (q_b^T emb_s) Wk = A_s Wk
  ctx_b = sum_s emb_s (Wv attn_{b,s}^T)^T = sum_s emb_s G_s^T

This cuts the big matmuls ~6x vs the direct form (1.34 GFLOP instead of
8.6 GFLOP per plane and per path).

Math per direction d (query sample b owned by this core, kv half KV):
  q_b = Q[b] @ Wq                                   [2048, 512]
  A^T[s*64+c, i] = sum_n KV[s,n,c] q_b[n,i]         [512, 512]
  scores[i, s*512+i'] = sum_c A^T[s*64+c, i] Wk[c,i']
  InstanceNorm over plane + softmax over j: the per-plane mean cancels in
  softmax, so only inv_sigma = rsqrt(var+eps) matters; scores*inv_sigma is
  bounded (|z| < ~7) so exp needs no row-max subtraction either.
  attn = exp(scores * inv_sigma); denom = rowsum (via exp accum_out)
  G^T[s*64+c, i] = sum_i' Wv[c,i'] attnT[(s,i'), i]  [512, 512]
  ctxT[i, n] = sum_{s,c} G^T[s*64+c, i] KV[s,n,c]    (/denom at eviction)
  out_b = ctxT^T @ Wo                                [2048, 64]

Sharding: core c handles query sample c of direction 0 (q=emb_l, kv=emb_u)
and query sample c of direction 1 (q=emb_u, kv=emb_l).  No collectives.

All matmul operands bf16 (fp32 PSUM accumulation); emb/weights converted
to bf16 on the host.  End-to-end rel-err ~6e-3 vs the 2e-2 gate.

Per-core structure, per direction:
  ph1:  q projection (row-tiled pairs of K=64 matmuls)
  ph2a: A^T, 4 PSUM banks (g = s-pairs) accumulating over 16 n-chunks
  ph2b: scores: single-shot K=64 row-tiled pair matmuls (s even/odd in
        AT partition halves); evictions split ACT/DVE; bn_stats on the
        bf16 scores for the plane variance
  ph3:  bn_aggr -> inv_sigma (cross-partition totals via PE ones matmul);
        ACT table pre-loads hide the Sqrt/Exp table switches
  ph4+5a (per s-pair): exp chunks [128,1024] (+denoms via accum_out),
        per-it transpose batches -> attnT, then G^T matmuls per s
  ph5b: ctxT = sum_g GT_g^T @ embSC_g per (i-chunk, n-block); 1/denom
        folded into the eviction
  ph6:  Wo projection, chunked output DMA
Direction 1's ph1+ph2a fill direction 0's scores-eviction/stats window;
direction 1's ph2b fills the gap before ph5b(d0).
"""

import sys

sys.path.insert(0, "/opt/trn_rl_repo")

import numpy as np
import ml_dtypes

_CACHE = {}
REPEAT = 1   # timing knob: execute the whole computation REPEAT times

B = 8          # samples per half-batch
N = 2048       # sequence length
C = 64         # embed dim
CH = 512       # head dim total (q/k/v channels)
J = B * CH     # 4096, kv concat width
NT = N // 128  # 16 n-tiles
IT = CH // 128  # 4 i-tiles
EPS = 1e-5


def _emit(nc, tc, bass, mybir, t):
    f32 = mybir.dt.float32
    bf16 = mybir.dt.bfloat16
    X = mybir.AxisListType
    AF = mybir.ActivationFunctionType
    ALU = mybir.AluOpType

    embT = t["embT"]    # [64, 16*2048] bf16, col = sglobal*2048+n
    embTq = t["embTq"]  # [64, 2*2048] bf16, per-core q samples (dir0, dir1)
    embN = t["embN"]    # [2048, 2*512] bf16, col = d*512 + s*64+c
    out_d = t["out"]    # [2, 2048, 64] f32

    embN_r = embN.rearrange("(k p) s -> p k s", p=128)  # [128, 16, 1024]

    with (
        tc.tile_pool(name="sb", bufs=1) as sb,
        tc.tile_pool(name="ps", bufs=8, space=bass.MemorySpace.PSUM) as ps,
    ):
        def acc():
            return ps.tile([128, 512], f32, tag="acc", bufs=4, name="acc")

        def bankw():
            # two-bank PSUM tile: row-tiled matmul pairs write the halves,
            # one 1024-wide op evicts (or exps) both
            return ps.tile([128, 1024], f32, tag="bankw", bufs=2,
                           name="bankw")

        # --- persistent weights (Wq/Wk duplicated into both partition
        # halves so row-tiled pair matmuls can source at base_partition 64)
        Wq_s = sb.tile([128, 512], bf16)
        nc.sync.dma_start(Wq_s[0:64, :], t["Wq"])
        nc.sync.dma_start(Wq_s[64:128, :], t["Wq"])

        eps_t = sb.tile([1, 1], f32, tag="eps_t")
        nc.gpsimd.memset(eps_t[:], float(EPS))
        ones_t = sb.tile([128, 1], f32, tag="ones_t")
        nc.gpsimd.memset(ones_t[:], 1.0)
        dmy = sb.tile([1, 2], f32, tag="dmy")
        nc.gpsimd.memset(dmy[:], 1.0)

        st = {}  # per-dir tile state

        def ph1(d):
            """q projection for dir d + embN prefetch for dir d."""
            qoff = d * N
            q_s = sb.tile([128, NT * 512], bf16, tag="q_s", name="q_s")
            eqs = sb.tile([128, N], bf16, tag="eqs", name="eqs")
            nc.sync.dma_start(eqs[0:64, :], embTq[:, qoff:qoff + N])
            nc.sync.dma_start(eqs[64:128, :], embTq[:, qoff:qoff + N])
            for nk in range(0, NT, 2):
                qw = bankw()
                nc.tensor.matmul(
                    qw[:, 0:512], eqs[0:64, nk * 128:(nk + 1) * 128],
                    Wq_s[0:64, :], start=True, stop=True,
                    tile_position=(0, 0),
                )
                nc.tensor.matmul(
                    qw[:, 512:1024], eqs[64:128, (nk + 1) * 128:(nk + 2) * 128],
                    Wq_s[64:128, :], start=True, stop=True,
                    tile_position=(64, 0),
                )
                if (nk >> 1) & 1:
                    nc.scalar.copy(q_s[:, nk * 512:(nk + 2) * 512], qw[:, :])
                else:
                    nc.vector.tensor_copy(q_s[:, nk * 512:(nk + 2) * 512],
                                          qw[:, :])
            st.setdefault(d, {})["q_s"] = q_s

        def prefetch_eN(d):
            """n-major kv embedding for ph2a(d): 2 MB DMA, issued early."""
            eN = sb.tile([128, NT * 512], bf16, tag="eN", bufs=2, name="eN")
            eN3 = eN[:].rearrange("p (k s) -> p k s", s=512)
            for k0 in range(0, NT, 4):   # chunked: ph2a starts on chunk 0
                nc.sync.dma_start(
                    eN3[:, k0:k0 + 4, :],
                    embN_r[:, k0:k0 + 4, d * 512:(d + 1) * 512],
                )
            st.setdefault(d, {})["eN"] = eN

        def ph2a(d):
            """A^T[g] = sum_n embN_chunk^T @ q: 4 banks over 16 n-chunks."""
            q_s, eN = st[d]["q_s"], st[d]["eN"]
            # pre-load the Sqrt ACT table off the critical path (Copy works
            # under any loaded set; the Exp table is pre-loaded in ph3)
            nc.scalar.activation(dmy[:, 1:2], dmy[:, 0:1], AF.Sqrt)
            ab = [acc() for _ in range(4)]
            for nk in range(NT):
                for g in range(4):
                    nc.tensor.matmul(
                        ab[g][:, :],
                        eN[:, nk * 512 + g * 128:nk * 512 + (g + 1) * 128],
                        q_s[:, nk * 512:(nk + 1) * 512],
                        start=(nk == 0), stop=(nk == NT - 1),
                    )
            AT = sb.tile([128, 4 * 512], bf16, tag="AT", bufs=2, name="AT")
            u = sb.tile([128, 4], f32, tag="u", name="u")
            for g in range(4):
                # u[sc] = sum_i A^T[sc, i] feeds the plane-mean path; split
                # engines so the L-matmuls can start after ~2 evictions
                if g < 2:
                    nc.scalar.activation(AT[:, g * 512:(g + 1) * 512],
                                         ab[g][:, :], AF.Copy,
                                         accum_out=u[:, g:g + 1])
                else:
                    nc.vector.tensor_copy(AT[:, g * 512:(g + 1) * 512],
                                          ab[g][:, :])
                    nc.vector.reduce_sum(u[:, g:g + 1], ab[g][:, :],
                                         axis=X.X)
            # B^T[(s,c), i] = sum_c' L[c',c] A^T[(s,c'), i]; sum B^2 = sum S^2
            # (K_hat = Wk Wk^T = L L^T, L from the host)
            bst = sb.tile([128, 4 * 6], f32, tag="bst", name="bst")
            for gp in range(2):
                bb = bankw()
                for gz in range(2):
                    g = gp * 2 + gz
                    for z in range(2):
                        nc.tensor.matmul(
                            bb[z * 64:(z + 1) * 64,
                               gz * 512:(gz + 1) * 512],
                            L_s[z * 64:(z + 1) * 64, :],
                            AT[z * 64:(z + 1) * 64, g * 512:(g + 1) * 512],
                            start=True, stop=True,
                            tile_position=(z * 64, z * 64),
                        )
                for gz in range(2):
                    g = gp * 2 + gz
                    nc.vector.bn_stats(bst[:, g * 6:(g + 1) * 6],
                                       bb[:, gz * 512:(gz + 1) * 512])
            st[d]["AT"] = AT
            st[d]["u"] = u
            st[d]["bst"] = bst

        def m245(d):
            """Per s-pair g: scores matmuls + evictions, exp chunks,
            transpose batches, and G^T — fully pipelined since inv_sigma
            comes from the B-statistics (no dependence on the scores)."""
            AT = st[d]["AT"]
            invb = st[d]["invb"]
            kvs = (1 - d) * B
            attn_s = sb.tile([128, IT * J], bf16, tag="A", name="attn_s")
            # attnT layout: col = jk*512 + it*128 + ii (jk-major)
            attnT_s = sb.tile([128, IT * J], bf16, tag="T", name="attnT_s")
            aT3 = attnT_s.rearrange("p (j c) -> p j c", c=512)
            dparts = sb.tile([128, IT * 4], f32, tag="dp", name="dparts")
            GT = sb.tile([128, 4 * 512], bf16, tag="GT", name="GT")
            esc = st[d]["esc"]

            def transposes_gt(g, its):
                s0 = 2 * g
                for it in its:
                    # 8 transposes (jk = 4*s0 .. 4*s0+7) batched per it
                    tp = ps.tile([128, 1024], bf16, tag="acc", bufs=4,
                                 name="tp")
                    for z in range(8):
                        jk = s0 * 4 + z
                        nc.tensor.transpose(
                            tp[:, z * 128:(z + 1) * 128],
                            attn_s[:, it * J + jk * 128:
                                   it * J + (jk + 1) * 128],
                            eye_s[:],
                        )
                    nc.vector.tensor_copy(
                        aT3[:, s0 * 4:s0 * 4 + 8, it * 128:(it + 1) * 128],
                        tp[:].rearrange("p (j c) -> p j c", c=128),
                    )

            def gt_mms(g):
                # G^T for 2g, 2g+1: accumulate into bank g's halves
                gbank = acc()
                for z in range(2):
                    s = 2 * g + z
                    for ip in range(4):
                        jk = s * 4 + ip
                        nc.tensor.matmul(
                            gbank[z * 64:(z + 1) * 64, :],
                            WvT_s[:, ip * 64:(ip + 1) * 64],
                            attnT_s[:, jk * 512:(jk + 1) * 512],
                            start=(ip == 0), stop=(ip == 3),
                            tile_position=(0, z * 64),
                        )
                # DVE only: keeps the ACT queue a pure exp stream here
                nc.vector.tensor_copy(GT[:, g * 512:(g + 1) * 512],
                                      gbank[:, :])

            def sc_exp(g, ic):
                """scores pair (ic; s=2g,2g+1) into a two-bank PSUM tile,
                exp'd straight out of PSUM (no SBUF scores at all)."""
                s0 = 2 * g
                bw = bankw()
                nc.tensor.matmul(
                    bw[:, 0:512], AT[0:64, g * 512 + ic * 128:
                                     g * 512 + (ic + 1) * 128],
                    Wk_s[0:64, :], start=True, stop=True,
                    tile_position=(0, 0),
                )
                nc.tensor.matmul(
                    bw[:, 512:1024], AT[64:128, g * 512 + ic * 128:
                                        g * 512 + (ic + 1) * 128],
                    Wk_s[64:128, :], start=True, stop=True,
                    tile_position=(64, 0),
                )
                nc.scalar.activation(
                    attn_s[:, ic * J + s0 * 512:ic * J + (s0 + 2) * 512],
                    bw[:, :], AF.Exp, scale=invb[:],
                    accum_out=dparts[:, ic * 4 + g:ic * 4 + g + 1],
                )

            # pipeline: transposes/G^T run one exp-group behind, split
            # around the mid-g score matmuls to absorb bankw rotation waits
            for g in range(4):
                if len(esc) < 4:  # finish the embSC prefetches for ph5b
                    esc.append(_load_esc(kvs, len(esc)))
                sc_exp(g, 0)
                sc_exp(g, 1)
                if g > 0:
                    transposes_gt(g - 1, (0, 1))
                sc_exp(g, 2)
                sc_exp(g, 3)
                if g > 0:
                    transposes_gt(g - 1, (2, 3))
                    gt_mms(g - 1)
            transposes_gt(3, (0, 1, 2, 3))
            gt_mms(3)

            denom = sb.tile([128, IT], f32, tag="denom")
            rden = sb.tile([128, IT], f32, tag="rden")
            for it in range(IT):
                nc.vector.reduce_sum(denom[:, it:it + 1],
                                     dparts[:, it * 4:(it + 1) * 4], axis=X.X)
            nc.vector.reciprocal(rden[:], denom[:])
            st[d]["GT"] = GT
            st[d]["rden"] = rden

        def _load_esc(kvs, g):
            """embSC tile g: samples 2g (rows 0-64) and 2g+1 (64-128)."""
            esc = sb.tile([128, N], bf16, tag="eS", bufs=4, name="esc")
            nc.sync.dma_start(
                esc[0:64, :],
                embT[:, (kvs + 2 * g) * N:(kvs + 2 * g + 1) * N])
            nc.sync.dma_start(
                esc[64:128, :],
                embT[:, (kvs + 2 * g + 1) * N:(kvs + 2 * g + 2) * N])
            return esc

        def ph3(d):
            """plane variance -> inv_sigma from the B-statistics (sum S^2 =
            sum B^2) and the u/wbar dot (sum S).  Exp bias is not needed
            (softmax shift-invariance) and |s*inv| < ~7 so no row-max."""
            u, bst = st[d]["u"], st[d]["bst"]
            kvs = (1 - d) * B
            esc = st[d].setdefault("esc", [])
            if d == 0:
                while len(esc) < 2:  # prefetch first embSC tiles for ph5b
                    esc.append(_load_esc(kvs, len(esc)))

            bsa = sb.tile([128, 2], f32, tag="cs", name="bsa")
            nc.vector.bn_aggr(bsa[:], bst[:])
            cstat = sb.tile([128, 2], f32, tag="cstat")
            # sumsqB_row = (varB + meanB^2) * 2048
            nc.vector.scalar_tensor_tensor(
                cstat[:, 1:2], bsa[:, 0:1], bsa[:, 0:1], bsa[:, 1:2],
                op0=ALU.mult, op1=ALU.add,
            )
            nc.vector.tensor_scalar_mul(cstat[:, 1:2], cstat[:, 1:2],
                                        float(4 * 512))
            # msum_row = sum_g u[r,g] * wbar[r]
            mp = sb.tile([128, 4], f32, tag="mp", name="mp")
            nc.vector.tensor_scalar(mp[:], u[:], wb_s[:], 1.0,
                                    op0=ALU.mult, op1=ALU.mult)
            nc.vector.reduce_sum(cstat[:, 0:1], mp[:], axis=X.X)
            # cross-partition totals via PE ones-vector matmul
            pst = bankw()
            nc.tensor.matmul(pst[:1, :2], ones_t[:], cstat[:],
                             start=True, stop=True)
            tstat = sb.tile([1, 2], f32, tag="tstat")
            nc.vector.tensor_copy(tstat[:], pst[:1, :2])
            mean = sb.tile([1, 1], f32, tag="mean")
            ex2 = sb.tile([1, 1], f32, tag="ex2")
            INVM = 1.0 / float(CH * J)
            nc.vector.tensor_scalar_mul(mean[:], tstat[:, 0:1], INVM)
            nc.vector.tensor_scalar_mul(ex2[:], tstat[:, 1:2], INVM)
            negvar = sb.tile([1, 1], f32, tag="negvar")
            nc.vector.scalar_tensor_tensor(
                negvar[:], mean[:], mean[:], ex2[:],
                op0=ALU.mult, op1=ALU.subtract,
            )
            sig = sb.tile([1, 1], f32, tag="sig")
            # sqrt(-negvar + eps) = sqrt(var + eps); table pre-loaded
            nc.scalar.activation(sig[:], negvar[:], AF.Sqrt,
                                 bias=eps_t[:], scale=-1.0)
            # pre-load the Exp table while DVE finishes the chain
            nc.scalar.activation(dmy[:, 1:2], dmy[:, 0:1], AF.Exp)
            inv = sb.tile([1, 1], f32, tag="inv")
            nc.vector.reciprocal(inv[:], sig[:])
            invb = sb.tile([128, 1], f32, tag="invb")
            nc.gpsimd.partition_broadcast(invb[:], inv[:])
            st[d]["invb"] = invb

        def ph5b(d):
            """ctxT[ic, nt] = sum_g GT_g(ic)^T @ embSC_g; /denom folded into
            the eviction."""
            GT, rden, esc = st[d]["GT"], st[d]["rden"], st[d]["esc"]
            ctxT_s = sb.tile([128, IT * 4 * 512], bf16, tag="A",
                             name="ctxT_s")
            for ic in range(IT):
                for nt in range(4):
                    cp = acc()
                    for g in range(4):
                        nc.tensor.matmul(
                            cp[:, :],
                            GT[:, g * 512 + ic * 128:g * 512 + (ic + 1) * 128],
                            esc[g][:, nt * 512:(nt + 1) * 512],
                            start=(g == 0), stop=(g == 3),
                        )
                    dst = ctxT_s[:, (ic * 4 + nt) * 512:
                                 (ic * 4 + nt + 1) * 512]
                    if nt & 1:
                        nc.vector.tensor_scalar_mul(dst, cp[:, :],
                                                    rden[:, ic:ic + 1])
                    else:
                        nc.scalar.activation(dst, cp[:, :], AF.Copy,
                                             scale=rden[:, ic:ic + 1])
            st[d]["ctxT_s"] = ctxT_s

        def ph6(d):
            """output projection + chunked DMA."""
            ctxT_s = st[d]["ctxT_s"]
            out_s = sb.tile([128, 16 * 64], f32, tag="out_s", name="out_s")
            odst = out_d[d].rearrange("(g p) c -> p g c", p=128)
            osrc = out_s[:].rearrange("p (g c) -> p g c", g=NT)
            for g in range(NT):
                nt, sub = g >> 2, g & 3
                op = acc()
                for it in range(IT):
                    nc.tensor.matmul(
                        op[:, :64],
                        ctxT_s[:, (it * 4 + nt) * 512 + sub * 128:
                               (it * 4 + nt) * 512 + (sub + 1) * 128],
                        Wo_s[:, it * 64:(it + 1) * 64],
                        start=(it == 0), stop=(it == IT - 1),
                    )
                # DVE only: ACT must stay free for the next dir's stats
                # chain and first exps
                nc.vector.tensor_copy(out_s[:, g * 64:(g + 1) * 64],
                                      op[:, :64])
                if g & 3 == 3:
                    nc.sync.dma_start(odst[:, g - 3:g + 1, :],
                                      osrc[:, g - 3:g + 1, :])

        import contextlib
        loop_cm = (tc.For_i(0, REPEAT, 1) if REPEAT > 1
                   else contextlib.nullcontext())
        with loop_cm:
            ph1(0)
            prefetch_eN(0)
            # remaining persistent weights, behind ph1's inputs on the queue
            Wk_s = sb.tile([128, 512], bf16)
            nc.sync.dma_start(Wk_s[0:64, :], t["Wk"])
            nc.sync.dma_start(Wk_s[64:128, :], t["Wk"])
            WvT_s = sb.tile([128, IT * 64], bf16)
            wvt_r = t["WvT"].rearrange("(k p) c -> k p c", p=128)
            for i in range(IT):
                nc.sync.dma_start(WvT_s[:, i * 64:(i + 1) * 64], wvt_r[i])
            Wo_s = sb.tile([128, IT * 64], bf16)
            wo_r = t["Wo"].rearrange("(t p) c -> t p c", p=128)
            for i in range(IT):
                nc.sync.dma_start(Wo_s[:, i * 64:(i + 1) * 64], wo_r[i])
            eye_s = sb.tile([128, 128], bf16)
            nc.sync.dma_start(eye_s[:], t["eye"])
            L_s = sb.tile([128, 64], bf16)
            nc.sync.dma_start(L_s[:], t["L"])
            wb_s = sb.tile([128, 1], f32)
            nc.sync.dma_start(wb_s[:], t["wb"])

            ph2a(0)
            ph3(0)     # stats need only ph2a outputs now
            prefetch_eN(1)
            m245(0)
            ph1(1)     # ACT/DVE drain m245's tail during these matmuls
            ph2a(1)
            ph5b(0)
            ph3(1)     # d1 stats chain runs under ph6(0)
            ph6(0)
            esc1 = st[1].setdefault("esc", [])
            while len(esc1) < 4:   # d1 embSC after d0's are consumed
                esc1.append(_load_esc(0 * B, len(esc1)))
            m245(1)
            ph5b(1)
            ph6(1)


def _build():
    key = ("nc", REPEAT)
    if key in _CACHE:
        return _CACHE[key]
    from concourse import bass, bacc, tile, mybir

    f32 = mybir.dt.float32
    bf16 = mybir.dt.bfloat16

    nc = bacc.Bacc("TRN2", target_bir_lowering=False, debug=False)
    t = {
        "embT": nc.dram_tensor("embT", [C, 2 * B * N], bf16,
                               kind="ExternalInput").ap(),
        "embTq": nc.dram_tensor("embTq", [C, 2 * N], bf16,
                                kind="ExternalInput").ap(),
        "embN": nc.dram_tensor("embN", [N, 2 * B * C], bf16,
                               kind="ExternalInput").ap(),
        "Wq": nc.dram_tensor("Wq", [C, CH], bf16, kind="ExternalInput").ap(),
        "Wk": nc.dram_tensor("Wk", [C, CH], bf16, kind="ExternalInput").ap(),
        "WvT": nc.dram_tensor("WvT", [CH, C], bf16,
                              kind="ExternalInput").ap(),
        "Wo": nc.dram_tensor("Wo", [CH, C], bf16, kind="ExternalInput").ap(),
        "eye": nc.dram_tensor("eye", [128, 128], bf16,
                              kind="ExternalInput").ap(),
        "L": nc.dram_tensor("L", [128, 64], bf16,
                            kind="ExternalInput").ap(),
        "wb": nc.dram_tensor("wb", [128, 1], f32,
                             kind="ExternalInput").ap(),
        "out": nc.dram_tensor("out", [2, N, C], f32,
                              kind="ExternalOutput").ap(),
    }
    with tile.TileContext(nc) as tc:
        _emit(nc, tc, bass, mybir, t)
    nc.compile()
    _CACHE[key] = nc
    return nc


def kernel(emb, Wq, Wk, Wv, Wo):
    from concourse.bass_utils import run_bass_kernel_spmd

    bf = ml_dtypes.bfloat16
    emb = np.asarray(emb, dtype=np.float32)
    Wq = np.ascontiguousarray(np.asarray(Wq, dtype=np.float32)).astype(bf)
    Wk = np.ascontiguousarray(np.asarray(Wk, dtype=np.float32)).astype(bf)
    WvT = np.ascontiguousarray(
        np.asarray(Wv, dtype=np.float32).T).astype(bf)
    Wo = np.ascontiguousarray(np.asarray(Wo, dtype=np.float32)).astype(bf)

    nc = _build()

    embT = np.ascontiguousarray(
        emb.transpose(2, 0, 1).reshape(C, 2 * B * N)).astype(bf)
    # embN[n, d*512 + s*64+c] = emb[kv_half(d) sample s, n, c]
    embN = np.empty((N, 2 * B * C), dtype=np.float32)
    embN[:, 0:B * C] = emb[B:].transpose(1, 0, 2).reshape(N, B * C)
    embN[:, B * C:] = emb[:B].transpose(1, 0, 2).reshape(N, B * C)
    embN = np.ascontiguousarray(embN).astype(bf)
    eye = np.eye(128, dtype=bf)
    # Cholesky factor of Wk Wk^T (for sum S^2 = ||A L||_F^2) and Wk row
    # sums (for sum S), both duplicated into the partition halves
    Wk32 = Wk.astype(np.float32)
    Khat = Wk32 @ Wk32.T
    Lc = np.linalg.cholesky(Khat + 1e-10 * np.eye(C))
    Ldup = np.ascontiguousarray(np.vstack([Lc, Lc])).astype(bf)
    wbar = Wk32.sum(1)
    wbdup = np.ascontiguousarray(
        np.concatenate([wbar, wbar])[:, None].astype(np.float32))

    in_maps = []
    for c in range(B):
        embTq = np.ascontiguousarray(
            np.concatenate([embT[:, c * N:(c + 1) * N],
                            embT[:, (B + c) * N:(B + c + 1) * N]], axis=1))
        in_maps.append({
            "embT": embT, "embTq": embTq, "embN": embN,
            "Wq": Wq, "Wk": Wk, "WvT": WvT, "Wo": Wo, "eye": eye,
            "L": Ldup, "wb": wbdup,
        })

    r = run_bass_kernel_spmd(nc, in_maps, list(range(B)))
    res = r.results

    out = np.empty((2 * B, N, C), dtype=np.float32)
    for c in range(B):
        out[c] = res[c]["out"][0]
        out[B + c] = res[c]["out"][1]
    return out


# revision 29
# speedup vs baseline: 1.4543x; 1.0114x over previous
"""Trainium2 Bass kernel for nn_Cross_Attention_Global (sparse_attention).

Key algebraic identity: the [512, 4096] score block per (query sample b,
kv sample s) is a product of rank-64 factors, so contract over the embed
dim c=64 FIRST:

  scores_{b,s} = q_b^T (emb_s Wk) = (q_b^T emb_s) Wk = A_s Wk
  ctx_b = sum_s emb_s (Wv attn_{b,s}^T)^T = sum_s emb_s G_s^T

This cuts the big matmuls ~6x vs the direct form (1.34 GFLOP instead of
8.6 GFLOP per plane and per path).

Math per direction d (query sample b owned by this core, kv half KV):
  q_b = Q[b] @ Wq                                   [2048, 512]
  A^T[s*64+c, i] = sum_n KV[s,n,c] q_b[n,i]         [512, 512]
  scores[i, s*512+i'] = sum_c A^T[s*64+c, i] Wk[c,i']
  InstanceNorm over plane + softmax over j: the per-plane mean cancels in
  softmax, so only inv_sigma = rsqrt(var+eps) matters; scores*inv_sigma is
  bounded (|z| < ~7) so exp needs no row-max subtraction either.  The
  plane moments come from A^T alone: sum S = <u, wbar> with u = A^T 1 and
  wbar = Wk 1, and sum S^2 = ||A L||_F^2 with Wk Wk^T = L L^T (host
  Cholesky) -- so inv_sigma never waits on the scores.
  attn = exp(scores * inv_sigma); denom = rowsum (via exp accum_out)
  G^T[s*64+c, i] = sum_i' Wv[c,i'] attnT[(s,i'), i]  [512, 512]
  ctxT[i, n] = sum_{s,c} G^T[s*64+c, i] KV[s,n,c]    (/denom at eviction)
  out_b = ctxT^T @ Wo                                [2048, 64]

Sharding: core c handles query sample c of direction 0 (q=emb_l, kv=emb_u)
and query sample c of direction 1 (q=emb_u, kv=emb_l).  No collectives.

All matmul operands bf16 (fp32 PSUM accumulation); emb/weights converted
to bf16 on the host.  End-to-end rel-err ~5e-3 vs the 2e-2 gate.

Per-core structure, per direction:
  ph1:  q projection (row-tiled pairs of K=64 matmuls into two-bank PSUM
        tiles, one 1024-wide eviction each)
  ph2a: A^T in 4 PSUM banks (g = s-pairs) accumulated over 16 n-chunks;
        evictions carry accum_out (u); then B = A L matmuls + bn_stats
        for the plane variance
  ph3:  bn_aggr + ones-matmul partition reduce -> inv_sigma; dummy
        activations pre-load the Sqrt/Exp ACT tables off-path
  m245 (per s-pair g): score pair matmuls into two-bank PSUM tiles,
        exp'd STRAIGHT OUT OF PSUM (scores never hit SBUF) with row
        denominators via accum_out; transposes and G^T run one exp-group
        behind so they never wait on the ACT stream
  ph5b: ctxT = sum_g GT_g^T @ embSC_g per (i-chunk, n-block); 1/denom
        folded into the eviction
  ph6:  Wo projection, chunked output DMA
Direction 1's ph1/ph2a/ph3 are emitted inside direction 0's stats and
ph5b windows so PE never waits on a serial chain.  Weight/embedding
inputs are host-packed (duplicated partition halves, folded embSC
layout) so each load is a single DMA descriptor.
"""

import sys

sys.path.insert(0, "/opt/trn_rl_repo")

import numpy as np
import ml_dtypes

_CACHE = {}
REPEAT = 1   # timing knob: execute the whole computation REPEAT times

B = 8          # samples per half-batch
N = 2048       # sequence length
C = 64         # embed dim
CH = 512       # head dim total (q/k/v channels)
J = B * CH     # 4096, kv concat width
NT = N // 128  # 16 n-tiles
IT = CH // 128  # 4 i-tiles
EPS = 1e-5


def _emit(nc, tc, bass, mybir, t):
    f32 = mybir.dt.float32
    bf16 = mybir.dt.bfloat16
    X = mybir.AxisListType
    AF = mybir.ActivationFunctionType
    ALU = mybir.AluOpType

    embP = t["embP"]    # [128, 8*2048] bf16: embSC tiles, one per (d, g)
    embTq = t["embTq"]  # [64, 2*2048] bf16, per-core q samples (dir0, dir1)
    embN = t["embN"]    # [2048, 2*512] bf16, col = d*512 + s*64+c
    out_d = t["out"]    # [2, 2048, 64] f32

    embN_r = embN.rearrange("(k p) s -> p k s", p=128)  # [128, 16, 1024]

    with (
        tc.tile_pool(name="sb", bufs=1) as sb,
        tc.tile_pool(name="ps", bufs=8, space=bass.MemorySpace.PSUM) as ps,
    ):
        def acc():
            return ps.tile([128, 512], f32, tag="acc", bufs=4, name="acc")

        def bankw():
            # two-bank PSUM tile: row-tiled matmul pairs write the halves,
            # one 1024-wide op evicts (or exps) both
            return ps.tile([128, 1024], f32, tag="bankw", bufs=2,
                           name="bankw")

        # --- persistent weights (Wq/Wk duplicated into both partition
        # halves so row-tiled pair matmuls can source at base_partition 64)
        Wq_s = sb.tile([128, 512], bf16)
        nc.sync.dma_start(Wq_s[:], t["Wq"])

        eps_t = sb.tile([1, 1], f32, tag="eps_t")
        nc.gpsimd.memset(eps_t[:], float(EPS))
        ones_t = sb.tile([128, 1], f32, tag="ones_t")
        nc.gpsimd.memset(ones_t[:], 1.0)
        dmy = sb.tile([1, 2], f32, tag="dmy")
        nc.gpsimd.memset(dmy[:], 1.0)
        # load the Sqrt ACT table immediately: ph1's Copy evictions run
        # under it, and the first real Sqrt (ph3) needs no further load
        nc.scalar.activation(dmy[:, 1:2], dmy[:, 0:1], AF.Sqrt)

        st = {}  # per-dir tile state

        def ph1(d):
            """q projection for dir d + embN prefetch for dir d."""
            qoff = d * N
            q_s = sb.tile([128, NT * 512], bf16, tag="q_s", name="q_s")
            eqs = sb.tile([128, N], bf16, tag="eqs", name="eqs")
            nc.sync.dma_start(eqs[:], embTq[:, qoff:qoff + N])
            for nk in range(0, NT, 2):
                qw = bankw()
                nc.tensor.matmul(
                    qw[:, 0:512], eqs[0:64, nk * 128:(nk + 1) * 128],
                    Wq_s[0:64, :], start=True, stop=True,
                    tile_position=(0, 0),
                )
                nc.tensor.matmul(
                    qw[:, 512:1024], eqs[64:128, (nk + 1) * 128:(nk + 2) * 128],
                    Wq_s[64:128, :], start=True, stop=True,
                    tile_position=(64, 0),
                )
                if (nk >> 1) & 1:
                    nc.scalar.copy(q_s[:, nk * 512:(nk + 2) * 512], qw[:, :])
                else:
                    nc.vector.tensor_copy(q_s[:, nk * 512:(nk + 2) * 512],
                                          qw[:, :])
            st.setdefault(d, {})["q_s"] = q_s

        def prefetch_eN(d):
            """n-major kv embedding for ph2a(d): 2 MB DMA, issued early."""
            eN = sb.tile([128, NT * 512], bf16, tag="eN", bufs=2, name="eN")
            eN3 = eN[:].rearrange("p (k s) -> p k s", s=512)
            for k0 in range(0, NT, 4):   # chunked: ph2a starts on chunk 0
                nc.sync.dma_start(
                    eN3[:, k0:k0 + 4, :],
                    embN_r[:, k0:k0 + 4, d * 512:(d + 1) * 512],
                )
            st.setdefault(d, {})["eN"] = eN

        def ph2a(d):
            """A^T[g] = sum_n embN_chunk^T @ q: 4 banks over 16 n-chunks."""
            q_s, eN = st[d]["q_s"], st[d]["eN"]
            # pre-load the Sqrt ACT table off the critical path (Copy works
            # under any loaded set; the Exp table is pre-loaded in ph3)
            nc.scalar.activation(dmy[:, 1:2], dmy[:, 0:1], AF.Sqrt)
            ab = [acc() for _ in range(4)]
            for nk in range(NT):
                for g in range(4):
                    nc.tensor.matmul(
                        ab[g][:, :],
                        eN[:, nk * 512 + g * 128:nk * 512 + (g + 1) * 128],
                        q_s[:, nk * 512:(nk + 1) * 512],
                        start=(nk == 0), stop=(nk == NT - 1),
                    )
            AT = sb.tile([128, 4 * 512], bf16, tag="AT", bufs=2, name="AT")
            u = sb.tile([128, 4], f32, tag="u", name="u")
            for g in range(4):
                # u[sc] = sum_i A^T[sc, i] feeds the plane-mean path; split
                # engines so the L-matmuls can start after ~2 evictions
                if g < 2:
                    nc.scalar.activation(AT[:, g * 512:(g + 1) * 512],
                                         ab[g][:, :], AF.Copy,
                                         accum_out=u[:, g:g + 1])
                else:
                    nc.vector.tensor_copy(AT[:, g * 512:(g + 1) * 512],
                                          ab[g][:, :])
                    nc.vector.reduce_sum(u[:, g:g + 1], ab[g][:, :],
                                         axis=X.X)
            # B^T[(s,c), i] = sum_c' L[c',c] A^T[(s,c'), i]; sum B^2 = sum S^2
            # (K_hat = Wk Wk^T = L L^T, L from the host)
            bst = sb.tile([128, 4 * 6], f32, tag="bst", name="bst")
            for gp in range(2):
                bb = bankw()
                for gz in range(2):
                    g = gp * 2 + gz
                    for z in range(2):
                        nc.tensor.matmul(
                            bb[z * 64:(z + 1) * 64,
                               gz * 512:(gz + 1) * 512],
                            L_s[z * 64:(z + 1) * 64, :],
                            AT[z * 64:(z + 1) * 64, g * 512:(g + 1) * 512],
                            start=True, stop=True,
                            tile_position=(z * 64, z * 64),
                        )
                for gz in range(2):
                    g = gp * 2 + gz
                    nc.vector.bn_stats(bst[:, g * 6:(g + 1) * 6],
                                       bb[:, gz * 512:(gz + 1) * 512])
            st[d]["AT"] = AT
            st[d]["u"] = u
            st[d]["bst"] = bst

        def m245(d):
            """Per s-pair g: scores matmuls + evictions, exp chunks,
            transpose batches, and G^T — fully pipelined since inv_sigma
            comes from the B-statistics (no dependence on the scores)."""
            AT = st[d]["AT"]
            invb = st[d]["invb"]
            attn_s = sb.tile([128, IT * J], bf16, tag="A", name="attn_s")
            # attnT layout: col = jk*512 + it*128 + ii (jk-major)
            attnT_s = sb.tile([128, IT * J], bf16, tag="T", name="attnT_s")
            aT3 = attnT_s.rearrange("p (j c) -> p j c", c=512)
            dparts = sb.tile([128, IT * 4], f32, tag="dp", name="dparts")
            GT = sb.tile([128, 4 * 512], bf16, tag="GT", name="GT")
            esc = st[d]["esc"]

            def transposes_gt(g, its):
                s0 = 2 * g
                for it in its:
                    # 8 transposes (jk = 4*s0 .. 4*s0+7) batched per it
                    tp = ps.tile([128, 1024], bf16, tag="acc", bufs=4,
                                 name="tp")
                    for z in range(8):
                        jk = s0 * 4 + z
                        nc.tensor.transpose(
                            tp[:, z * 128:(z + 1) * 128],
                            attn_s[:, it * J + jk * 128:
                                   it * J + (jk + 1) * 128],
                            eye_s[:],
                        )
                    nc.vector.tensor_copy(
                        aT3[:, s0 * 4:s0 * 4 + 8, it * 128:(it + 1) * 128],
                        tp[:].rearrange("p (j c) -> p j c", c=128),
                    )

            def gt_mms(g):
                # G^T for 2g, 2g+1: accumulate into bank g's halves
                gbank = acc()
                for z in range(2):
                    s = 2 * g + z
                    for ip in range(4):
                        jk = s * 4 + ip
                        nc.tensor.matmul(
                            gbank[z * 64:(z + 1) * 64, :],
                            WvT_s[:, ip * 64:(ip + 1) * 64],
                            attnT_s[:, jk * 512:(jk + 1) * 512],
                            start=(ip == 0), stop=(ip == 3),
                            tile_position=(0, z * 64),
                        )
                # DVE only: keeps the ACT queue a pure exp stream here
                nc.vector.tensor_copy(GT[:, g * 512:(g + 1) * 512],
                                      gbank[:, :])

            def sc_exp(g, ic):
                """scores pair (ic; s=2g,2g+1) into a two-bank PSUM tile,
                exp'd straight out of PSUM (no SBUF scores at all)."""
                s0 = 2 * g
                bw = bankw()
                nc.tensor.matmul(
                    bw[:, 0:512], AT[0:64, g * 512 + ic * 128:
                                     g * 512 + (ic + 1) * 128],
                    Wk_s[0:64, :], start=True, stop=True,
                    tile_position=(0, 0),
                )
                nc.tensor.matmul(
                    bw[:, 512:1024], AT[64:128, g * 512 + ic * 128:
                                        g * 512 + (ic + 1) * 128],
                    Wk_s[64:128, :], start=True, stop=True,
                    tile_position=(64, 0),
                )
                nc.scalar.activation(
                    attn_s[:, ic * J + s0 * 512:ic * J + (s0 + 2) * 512],
                    bw[:, :], AF.Exp, scale=invb[:],
                    accum_out=dparts[:, ic * 4 + g:ic * 4 + g + 1],
                )

            # pipeline: transposes/G^T run one exp-group behind, split
            # around the mid-g score matmuls to absorb bankw rotation waits
            for g in range(4):
                if len(esc) < 4:  # finish the embSC prefetches for ph5b
                    esc.append(_load_esc(d, len(esc)))
                sc_exp(g, 0)
                sc_exp(g, 1)
                if g > 0:
                    transposes_gt(g - 1, (0, 1))
                sc_exp(g, 2)
                sc_exp(g, 3)
                if g > 0:
                    transposes_gt(g - 1, (2, 3))
                    gt_mms(g - 1)
            transposes_gt(3, (0, 1, 2, 3))
            gt_mms(3)

            denom = sb.tile([128, IT], f32, tag="denom")
            rden = sb.tile([128, IT], f32, tag="rden")
            for it in range(IT):
                nc.vector.reduce_sum(denom[:, it:it + 1],
                                     dparts[:, it * 4:(it + 1) * 4], axis=X.X)
            nc.vector.reciprocal(rden[:], denom[:])
            st[d]["GT"] = GT
            st[d]["rden"] = rden

        def _load_esc(d, g):
            """embSC tile g of dir d: samples 2g (rows 0-64), 2g+1
            (64-128), pre-folded on the host -> one contiguous DMA."""
            esc = sb.tile([128, N], bf16, tag="eS", bufs=4, name="esc")
            nc.sync.dma_start(esc[:],
                              embP[:, (d * 4 + g) * N:(d * 4 + g + 1) * N])
            return esc

        def ph3(d):
            """plane variance -> inv_sigma from the B-statistics (sum S^2 =
            sum B^2) and the u/wbar dot (sum S).  Exp bias is not needed
            (softmax shift-invariance) and |s*inv| < ~7 so no row-max."""
            u, bst = st[d]["u"], st[d]["bst"]
            esc = st[d].setdefault("esc", [])
            if d == 0:
                while len(esc) < 2:  # prefetch first embSC tiles for ph5b
                    esc.append(_load_esc(d, len(esc)))

            bsa = sb.tile([128, 2], f32, tag="cs", name="bsa")
            nc.vector.bn_aggr(bsa[:], bst[:])
            cstat = sb.tile([128, 2], f32, tag="cstat")
            # sumsqB_row = (varB + meanB^2) * 2048
            nc.vector.scalar_tensor_tensor(
                cstat[:, 1:2], bsa[:, 0:1], bsa[:, 0:1], bsa[:, 1:2],
                op0=ALU.mult, op1=ALU.add,
            )
            nc.vector.tensor_scalar_mul(cstat[:, 1:2], cstat[:, 1:2],
                                        float(4 * 512))
            # msum_row = sum_g u[r,g] * wbar[r]
            mp = sb.tile([128, 4], f32, tag="mp", name="mp")
            nc.vector.tensor_scalar(mp[:], u[:], wb_s[:], 1.0,
                                    op0=ALU.mult, op1=ALU.mult)
            nc.vector.reduce_sum(cstat[:, 0:1], mp[:], axis=X.X)
            # cross-partition totals via PE ones-vector matmul
            pst = bankw()
            nc.tensor.matmul(pst[:1, :2], ones_t[:], cstat[:],
                             start=True, stop=True)
            tstat = sb.tile([1, 2], f32, tag="tstat")
            nc.vector.tensor_copy(tstat[:], pst[:1, :2])
            mean = sb.tile([1, 1], f32, tag="mean")
            ex2 = sb.tile([1, 1], f32, tag="ex2")
            INVM = 1.0 / float(CH * J)
            nc.vector.tensor_scalar_mul(mean[:], tstat[:, 0:1], INVM)
            nc.vector.tensor_scalar_mul(ex2[:], tstat[:, 1:2], INVM)
            negvar = sb.tile([1, 1], f32, tag="negvar")
            nc.vector.scalar_tensor_tensor(
                negvar[:], mean[:], mean[:], ex2[:],
                op0=ALU.mult, op1=ALU.subtract,
            )
            sig = sb.tile([1, 1], f32, tag="sig")
            # sqrt(-negvar + eps) = sqrt(var + eps); table pre-loaded
            nc.scalar.activation(sig[:], negvar[:], AF.Sqrt,
                                 bias=eps_t[:], scale=-1.0)
            # pre-load the Exp table while DVE finishes the chain
            nc.scalar.activation(dmy[:, 1:2], dmy[:, 0:1], AF.Exp)
            inv = sb.tile([1, 1], f32, tag="inv")
            nc.vector.reciprocal(inv[:], sig[:])
            invb = sb.tile([128, 1], f32, tag="invb")
            nc.gpsimd.partition_broadcast(invb[:], inv[:])
            st[d]["invb"] = invb

        def ph5b(d):
            """ctxT[ic, nt] = sum_g GT_g(ic)^T @ embSC_g; /denom folded into
            the eviction."""
            GT, rden, esc = st[d]["GT"], st[d]["rden"], st[d]["esc"]
            ctxT_s = sb.tile([128, IT * 4 * 512], bf16, tag="A",
                             name="ctxT_s")
            for ic in range(IT):
                for nt in range(4):
                    cp = acc()
                    for g in range(4):
                        nc.tensor.matmul(
                            cp[:, :],
                            GT[:, g * 512 + ic * 128:g * 512 + (ic + 1) * 128],
                            esc[g][:, nt * 512:(nt + 1) * 512],
                            start=(g == 0), stop=(g == 3),
                        )
                    dst = ctxT_s[:, (ic * 4 + nt) * 512:
                                 (ic * 4 + nt + 1) * 512]
                    # DVE only: ACT must be free for the next dir's
                    # Sqrt/Exp table loads and first exps
                    nc.vector.tensor_scalar_mul(dst, cp[:, :],
                                                rden[:, ic:ic + 1])
            st[d]["ctxT_s"] = ctxT_s

        def ph6(d):
            """output projection + chunked DMA."""
            ctxT_s = st[d]["ctxT_s"]
            out_s = sb.tile([128, 16 * 64], f32, tag="out_s", name="out_s")
            odst = out_d[d].rearrange("(g p) c -> p g c", p=128)
            osrc = out_s[:].rearrange("p (g c) -> p g c", g=NT)
            for g2 in range(0, NT, 2):
                op = acc()
                for gg in (g2, g2 + 1):
                    nt, sub = gg >> 2, gg & 3
                    off = (gg & 1) * 64
                    for it in range(IT):
                        nc.tensor.matmul(
                            op[:, off:off + 64],
                            ctxT_s[:, (it * 4 + nt) * 512 + sub * 128:
                                   (it * 4 + nt) * 512 + (sub + 1) * 128],
                            Wo_s[:, it * 64:(it + 1) * 64],
                            start=(it == 0), stop=(it == IT - 1),
                        )
                # DVE only (ACT stays free for the next dir's table loads);
                # two output groups per bank -> one eviction
                nc.vector.tensor_copy(out_s[:, g2 * 64:(g2 + 2) * 64],
                                      op[:, :128])
                if g2 & 3 == 2:
                    nc.sync.dma_start(odst[:, g2 - 2:g2 + 2, :],
                                      osrc[:, g2 - 2:g2 + 2, :])

        import contextlib
        loop_cm = (tc.For_i(0, REPEAT, 1) if REPEAT > 1
                   else contextlib.nullcontext())
        with loop_cm:
            ph1(0)
            prefetch_eN(0)
            # remaining persistent weights, behind ph1's inputs on the queue
            Wk_s = sb.tile([128, 512], bf16)
            nc.sync.dma_start(Wk_s[:], t["Wk"])
            WvT_s = sb.tile([128, IT * 64], bf16)
            nc.sync.dma_start(WvT_s[:], t["WvT"])
            Wo_s = sb.tile([128, IT * 64], bf16)
            nc.sync.dma_start(Wo_s[:], t["Wo"])
            eye_s = sb.tile([128, 128], bf16)
            nc.sync.dma_start(eye_s[:], t["eye"])
            L_s = sb.tile([128, 64], bf16)
            nc.sync.dma_start(L_s[:], t["L"])
            wb_s = sb.tile([128, 1], f32)
            nc.sync.dma_start(wb_s[:], t["wb"])

            ph2a(0)
            ph3(0)     # stats need only ph2a outputs now
            prefetch_eN(1)
            m245(0)
            ph1(1)     # ACT/DVE drain m245's tail during these matmuls
            ph2a(1)
            ph3(1)     # d1 stats + ACT table loads run under ph5b(0)
            ph5b(0)
            ph6(0)
            esc1 = st[1].setdefault("esc", [])
            while len(esc1) < 4:   # d1 embSC after d0's are consumed
                esc1.append(_load_esc(1, len(esc1)))
            m245(1)
            ph5b(1)
            ph6(1)


def _build():
    key = ("nc", REPEAT)
    if key in _CACHE:
        return _CACHE[key]
    from concourse import bass, bacc, tile, mybir

    f32 = mybir.dt.float32
    bf16 = mybir.dt.bfloat16

    nc = bacc.Bacc("TRN2", target_bir_lowering=False, debug=False)
    t = {
        "embP": nc.dram_tensor("embP", [2 * C, B * N], bf16,
                               kind="ExternalInput").ap(),
        "embTq": nc.dram_tensor("embTq", [2 * C, 2 * N], bf16,
                                kind="ExternalInput").ap(),
        "embN": nc.dram_tensor("embN", [N, 2 * B * C], bf16,
                               kind="ExternalInput").ap(),
        "Wq": nc.dram_tensor("Wq", [2 * C, CH], bf16,
                             kind="ExternalInput").ap(),
        "Wk": nc.dram_tensor("Wk", [2 * C, CH], bf16,
                             kind="ExternalInput").ap(),
        "WvT": nc.dram_tensor("WvT", [128, IT * C], bf16,
                              kind="ExternalInput").ap(),
        "Wo": nc.dram_tensor("Wo", [128, IT * C], bf16,
                             kind="ExternalInput").ap(),
        "eye": nc.dram_tensor("eye", [128, 128], bf16,
                              kind="ExternalInput").ap(),
        "L": nc.dram_tensor("L", [128, 64], bf16,
                            kind="ExternalInput").ap(),
        "wb": nc.dram_tensor("wb", [128, 1], f32,
                             kind="ExternalInput").ap(),
        "out": nc.dram_tensor("out", [2, N, C], f32,
                              kind="ExternalOutput").ap(),
    }
    with tile.TileContext(nc) as tc:
        _emit(nc, tc, bass, mybir, t)
    nc.compile()
    _CACHE[key] = nc
    return nc


def kernel(emb, Wq, Wk, Wv, Wo):
    from concourse.bass_utils import run_bass_kernel_spmd

    bf = ml_dtypes.bfloat16
    emb = np.asarray(emb, dtype=np.float32)
    Wq = np.asarray(Wq, dtype=np.float32).astype(bf)
    Wq = np.ascontiguousarray(np.vstack([Wq, Wq]))   # duplicated halves
    Wk = np.asarray(Wk, dtype=np.float32).astype(bf)
    Wkd = np.ascontiguousarray(np.vstack([Wk, Wk]))
    WvT = np.asarray(Wv, dtype=np.float32).T.astype(bf)
    WvT = np.ascontiguousarray(
        WvT.reshape(4, 128, C).transpose(1, 0, 2).reshape(128, 4 * C))
    Wo = np.asarray(Wo, dtype=np.float32).astype(bf)
    Wo = np.ascontiguousarray(
        Wo.reshape(4, 128, C).transpose(1, 0, 2).reshape(128, 4 * C))

    nc = _build()

    embT = np.ascontiguousarray(
        emb.transpose(2, 0, 1).reshape(C, 2 * B * N)).astype(bf)
    # embP[z*64+c, (d*4+g)*N + n] = emb[kv(d) sample 2g+z, n, c]
    embP = np.empty((2 * C, B * N), dtype=np.float32)
    for d in range(2):
        kvs = (1 - d) * B
        for g in range(4):
            for z in range(2):
                embP[z * C:(z + 1) * C, (d * 4 + g) * N:(d * 4 + g + 1) * N] \
                    = emb[kvs + 2 * g + z].T
    embP = np.ascontiguousarray(embP).astype(bf)
    # embN[n, d*512 + s*64+c] = emb[kv_half(d) sample s, n, c]
    embN = np.empty((N, 2 * B * C), dtype=np.float32)
    embN[:, 0:B * C] = emb[B:].transpose(1, 0, 2).reshape(N, B * C)
    embN[:, B * C:] = emb[:B].transpose(1, 0, 2).reshape(N, B * C)
    embN = np.ascontiguousarray(embN).astype(bf)
    eye = np.eye(128, dtype=bf)
    # Cholesky factor of Wk Wk^T (for sum S^2 = ||A L||_F^2) and Wk row
    # sums (for sum S), both duplicated into the partition halves
    Wk32 = Wk.astype(np.float32)
    Khat = Wk32 @ Wk32.T
    Lc = np.linalg.cholesky(Khat + 1e-10 * np.eye(C))
    Ldup = np.ascontiguousarray(np.vstack([Lc, Lc])).astype(bf)
    wbar = Wk32.sum(1)
    wbdup = np.ascontiguousarray(
        np.concatenate([wbar, wbar])[:, None].astype(np.float32))

    in_maps = []
    for c in range(B):
        half = np.concatenate([embT[:, c * N:(c + 1) * N],
                               embT[:, (B + c) * N:(B + c + 1) * N]], axis=1)
        embTq = np.ascontiguousarray(np.vstack([half, half]))
        in_maps.append({
            "embP": embP, "embTq": embTq, "embN": embN,
            "Wq": Wq, "Wk": Wkd, "WvT": WvT, "Wo": Wo, "eye": eye,
            "L": Ldup, "wb": wbdup,
        })

    r = run_bass_kernel_spmd(nc, in_maps, list(range(B)))
    res = r.results

    out = np.empty((2 * B, N, C), dtype=np.float32)
    for c in range(B):
        out[c] = res[c]["out"][0]
        out[B + c] = res[c]["out"][1]
    return out
